# revision 1
# baseline (speedup 1.0000x reference)
"""MLA (multi-head latent attention) prefill kernel for 8 TRN2 NeuronCores.

Sharding: 4 head-groups x 2 batches. Core c: batch = c // 4, head-group g = c % 4
(4 heads each). Each core computes its batch's down-projections + RMSNorm,
its 4 heads' q_up / attention / ctx, and a partial output projection
(out_w column slice). Host sums the 4 partials per batch (TP unshard).

Device compute uses fp32 storage; matmuls run as float32r (full-rate PE).
All weight transposes are done on host (numpy) so the device only transposes
activations (PE transpose via identity).
"""

import sys
import os

for _p in ("/opt/trn_rl_repo", "/root/.axon_site/_ro/trn_rl_repo"):
    if os.path.isdir(_p) and _p not in sys.path:
        sys.path.insert(0, _p)

import numpy as np

import concourse.bass as bass
import concourse.bacc as bacc
import concourse.tile as tile
import concourse.mybir as mybir
from concourse.bass_utils import run_bass_kernel_spmd
from concourse.masks import make_identity

F32 = mybir.dt.float32
F32R = mybir.dt.float32r

DIM, H, Q_RANK, KV_RANK = 2048, 16, 768, 512
QK_STATIC, QK_ROT, V_DIM = 128, 64, 128
QK_TOTAL = QK_STATIC + QK_ROT
BS, SEQ = 2, 1024
HPC = 4          # heads per core
N_CORES = 8
P = 128
NSB = SEQ // P   # 8 s-blocks
NKD = DIM // P   # 16 d-chunks

MM_F32R = True   # bitcast matmul operands to float32r (1 cyc/row vs 4)


MMDT = F32R if MM_F32R else F32


def _mm(ap):
    return ap


def build_kernel():
    nc = bacc.Bacc("TRN2", target_bir_lowering=False, debug=False)

    def din(name, shape, dt=F32):
        return nc.dram_tensor(name, list(shape), dt, kind="ExternalInput")

    xT = din("xT", (DIM, SEQ), MMDT)
    mask = din("mask", (SEQ, SEQ))
    wqdT = din("wqdT", (DIM, Q_RANK), MMDT)
    qdb = din("qdb", (1, Q_RANK), MMDT)
    wkvdT = din("wkvdT", (DIM, KV_RANK + QK_ROT), MMDT)
    kvdb = din("kvdb", (1, KV_RANK + QK_ROT), MMDT)
    wqupT = din("wqupT", (Q_RANK, HPC * QK_TOTAL), MMDT)
    qub = din("qub", (1, HPC * QK_TOTAL), MMDT)
    wkT = din("wkT", (HPC, KV_RANK, QK_STATIC), MMDT)
    wvT = din("wvT", (HPC, KV_RANK, V_DIM), MMDT)
    woutT = din("woutT", (HPC * V_DIM, DIM), MMDT)
    outb = din("outb", (1, DIM), MMDT)
    cosf = din("cosf", (SEQ, QK_ROT))
    sinhr = din("sinhr", (SEQ, QK_ROT))
    ones_in = din("ones_in", (1, P), MMDT)

    out_p = nc.dram_tensor("out_p", [SEQ, DIM], F32, kind="ExternalOutput")

    RV = KV_RANK + QK_ROT  # 576

    with tile.TileContext(nc) as tc:
        import contextlib
        ctx = contextlib.ExitStack()
        with ctx:
            const = ctx.enter_context(tc.tile_pool(name="const", bufs=1))
            persist = ctx.enter_context(tc.tile_pool(name="persist", bufs=1))
            scv = ctx.enter_context(tc.tile_pool(name="scratch_vec", bufs=4))
            ppt = ctx.enter_context(tc.tile_pool(name="psum_t", bufs=2, space="PSUM"))

            ident = const.tile([P, P], F32, tag="ident")
            make_identity(nc, ident[:])
            ones_row = const.tile([1, P], MMDT, tag="ones")
            nc.sync.dma_start(ones_row[:], ones_in[:])

            def load_const(name, src, shape, dt=F32):
                t = const.tile(list(shape), dt, name=name, tag=name)
                nc.sync.dma_start(t[:], src[:])
                return t

            t_qdb = load_const("qdb", qdb, (1, Q_RANK), MMDT)
            t_kvdb = load_const("kvdb", kvdb, (1, RV), MMDT)
            t_qub = load_const("qub", qub, (1, HPC * QK_TOTAL), MMDT)
            t_outb = load_const("outb", outb, (1, DIM), MMDT)

            t_cos, t_sin = [], []
            for sb in range(NSB):
                c = const.tile([P, QK_ROT], F32, name=f"cos{sb}", tag=f"cos{sb}")
                s = const.tile([P, QK_ROT], F32, name=f"sin{sb}", tag=f"sin{sb}")
                nc.sync.dma_start(c[:], cosf[sb * P:(sb + 1) * P, :])
                nc.sync.dma_start(s[:], sinhr[sb * P:(sb + 1) * P, :])
                t_cos.append(c)
                t_sin.append(s)

            # persistent activation tensors (per-partition: 4+16+4+16+16 = 56KB)
            kvnT = [persist.tile([P, SEQ], MMDT, name=f"kvnT{ct}", tag=f"kvnT{ct}")
                    for ct in range(4)]
            krT = persist.tile([QK_ROT, SEQ], MMDT, name="krT", tag="krT")
            qsT = [persist.tile([P, SEQ], MMDT, name=f"qsT{h}", tag=f"qsT{h}")
                   for h in range(HPC)]
            qrT = [persist.tile([QK_ROT, SEQ], MMDT, name=f"qrT{h}", tag=f"qrT{h}")
                   for h in range(HPC)]
            ctxT = [persist.tile([P, SEQ], MMDT, name=f"ctxT{h}", tag=f"ctxT{h}")
                    for h in range(HPC)]

            def rmsnorm_stats(pool, ps_list, widths, inv_n):
                ssqs = []
                for psrc, w in zip(ps_list, widths):
                    sq = pool.tile([P, w], F32, name="sq", tag="sq")
                    ssq = scv.tile([P, 1], F32, name="ssq", tag="ssq")
                    nc.scalar.activation(sq[:], psrc, mybir.ActivationFunctionType.Square,
                                         accum_out=ssq[:])
                    ssqs.append(ssq)
                tot = ssqs[0]
                if len(ssqs) > 1:
                    tot = scv.tile([P, 1], F32, name="ssq_tot", tag="ssq_tot")
                    nc.vector.tensor_tensor(tot[:], ssqs[0][:], ssqs[1][:],
                                            op=mybir.AluOpType.add)
                mseps = scv.tile([P, 1], F32, name="mseps", tag="mseps")
                nc.vector.tensor_scalar(mseps[:], tot[:], inv_n, 1e-6,
                                        op0=mybir.AluOpType.mult,
                                        op1=mybir.AluOpType.add)
                rinv = scv.tile([P, 1], F32, name="rinv", tag="rinv")
                nc.vector.reciprocal(rinv[:], mseps[:])
                rstd = scv.tile([P, 1], F32, name="rstd", tag="rstd")
                nc.scalar.sqrt(rstd[:], rinv[:])
                return rstd

            def rope(pool, dst, src_ap, sb):
                lo, hi = (0, 32), (32, 64)
                for (a0, a1), (b0, b1) in ((lo, hi), (hi, lo)):
                    m1 = pool.tile([P, 32], F32, name="rope_m1", tag="rope_m1")
                    m2 = pool.tile([P, 32], F32, name="rope_m2", tag="rope_m2")
                    nc.vector.tensor_tensor(m1[:], src_ap[:, a0:a1], t_cos[sb][:, a0:a1],
                                            op=mybir.AluOpType.mult)
                    nc.vector.tensor_tensor(m2[:], src_ap[:, b0:b1], t_sin[sb][:, a0:a1],
                                            op=mybir.AluOpType.mult)
                    nc.vector.tensor_tensor(dst[:, a0:a1], m1[:], m2[:],
                                            op=mybir.AluOpType.add)

            def transpose_to(dst_ap, src_ap, rows, cols):
                pst = ppt.tile([cols, rows], F32, name="trans", tag="trans")
                nc.tensor.transpose(pst[:], src_ap, ident[:rows, :rows])
                nc.vector.tensor_copy(dst_ap, pst[:])

            def load_x_slice(pool, sb):
                xs = pool.tile([P, NKD * P], MMDT, name="xsl", tag="xsl")
                for k in range(NKD):
                    nc.sync.dma_start(xs[:, k * P:(k + 1) * P],
                                      xT[k * P:(k + 1) * P, sb * P:(sb + 1) * P])
                return xs

            # ---------- PHASE 1: kv path ----------
            with tc.tile_pool(name="wkv_pool", bufs=1) as wp1, \
                 tc.tile_pool(name="sc1", bufs=2) as sc1, \
                 tc.tile_pool(name="pp1", bufs=4, space="PSUM") as pp:
                wkv = []
                for k in range(NKD):
                    t = wp1.tile([P, RV], MMDT, name=f"wkvd{k}", tag=f"wkvd{k}")
                    nc.sync.dma_start(t[:], wkvdT[k * P:(k + 1) * P, :])
                    wkv.append(t)

                for sb in range(NSB):
                    xs = load_x_slice(sc1, sb)
                    psA = pp.tile([P, KV_RANK], F32, name="ps_kvA", tag="ps")
                    psB = pp.tile([P, QK_ROT], F32, name="ps_kvB", tag="ps")
                    for k in range(NKD):
                        nc.tensor.matmul(psA[:], _mm(xs[:, k * P:(k + 1) * P]),
                                         _mm(wkv[k][:, :KV_RANK]),
                                         start=(k == 0), stop=False)
                        nc.tensor.matmul(psB[:], _mm(xs[:, k * P:(k + 1) * P]),
                                         _mm(wkv[k][:, KV_RANK:]),
                                         start=(k == 0), stop=False)
                    nc.tensor.matmul(psA[:], _mm(ones_row[:]), _mm(t_kvdb[:, :KV_RANK]),
                                     start=False, stop=True)
                    nc.tensor.matmul(psB[:], _mm(ones_row[:]), _mm(t_kvdb[:, KV_RANK:]),
                                     start=False, stop=True)

                    rstd = rmsnorm_stats(sc1, [psA[:]], [KV_RANK], 1.0 / KV_RANK)
                    kvn = sc1.tile([P, KV_RANK], F32, name="kvn", tag="kvn")
                    nc.vector.tensor_scalar(kvn[:], psA[:], rstd[:], None,
                                            op0=mybir.AluOpType.mult)
                    for ct in range(4):
                        transpose_to(kvnT[ct][:, sb * P:(sb + 1) * P],
                                     kvn[:, ct * P:(ct + 1) * P], P, P)
                    kr = sc1.tile([P, QK_ROT], F32, name="kr", tag="kr")
                    rope(sc1, kr, psB, sb)
                    transpose_to(krT[:, sb * P:(sb + 1) * P], kr[:], P, QK_ROT)

            # ---------- PHASE 2: q path ----------
            with tc.tile_pool(name="wq_pool", bufs=1) as wp2, \
                 tc.tile_pool(name="sc2", bufs=2) as sc2, \
                 tc.tile_pool(name="pp2", bufs=4, space="PSUM") as pp:
                wqd = []
                for k in range(NKD):
                    t = wp2.tile([P, Q_RANK], MMDT, name=f"wqd{k}", tag=f"wqd{k}")
                    nc.sync.dma_start(t[:], wqdT[k * P:(k + 1) * P, :])
                    wqd.append(t)
                wqu = []
                for k in range(Q_RANK // P):
                    t = wp2.tile([P, HPC * QK_TOTAL], MMDT, name=f"wqu{k}", tag=f"wqu{k}")
                    nc.sync.dma_start(t[:], wqupT[k * P:(k + 1) * P, :])
                    wqu.append(t)

                for sb in range(NSB):
                    xs = load_x_slice(sc2, sb)
                    psA = pp.tile([P, 512], F32, name="ps_qA", tag="ps")
                    psB = pp.tile([P, Q_RANK - 512], F32, name="ps_qB", tag="ps")
                    for k in range(NKD):
                        nc.tensor.matmul(psA[:], _mm(xs[:, k * P:(k + 1) * P]),
                                         _mm(wqd[k][:, :512]), start=(k == 0), stop=False)
                        nc.tensor.matmul(psB[:], _mm(xs[:, k * P:(k + 1) * P]),
                                         _mm(wqd[k][:, 512:]), start=(k == 0), stop=False)
                    nc.tensor.matmul(psA[:], _mm(ones_row[:]), _mm(t_qdb[:, :512]),
                                     start=False, stop=True)
                    nc.tensor.matmul(psB[:], _mm(ones_row[:]), _mm(t_qdb[:, 512:]),
                                     start=False, stop=True)

                    rstd = rmsnorm_stats(sc2, [psA[:], psB[:]], [512, Q_RANK - 512],
                                         1.0 / Q_RANK)
                    qn = sc2.tile([P, Q_RANK], F32, name="qn", tag="qn")
                    nc.vector.tensor_scalar(qn[:, :512], psA[:], rstd[:], None,
                                            op0=mybir.AluOpType.mult)
                    nc.vector.tensor_scalar(qn[:, 512:], psB[:], rstd[:], None,
                                            op0=mybir.AluOpType.mult)

                    qnT = []
                    for k in range(Q_RANK // P):
                        t = sc2.tile([P, P], MMDT, name=f"qnT{k}", tag=f"qnT{k}")
                        transpose_to(t[:], qn[:, k * P:(k + 1) * P], P, P)
                        qnT.append(t)

                    NQ = HPC * QK_TOTAL  # 768
                    psC = pp.tile([P, 512], F32, name="ps_quA", tag="ps")
                    psD = pp.tile([P, NQ - 512], F32, name="ps_quB", tag="ps")
                    for k in range(Q_RANK // P):
                        nc.tensor.matmul(psC[:], _mm(qnT[k][:]), _mm(wqu[k][:, :512]),
                                         start=(k == 0), stop=False)
                        nc.tensor.matmul(psD[:], _mm(qnT[k][:]), _mm(wqu[k][:, 512:]),
                                         start=(k == 0), stop=False)
                    nc.tensor.matmul(psC[:], _mm(ones_row[:]), _mm(t_qub[:, :512]),
                                     start=False, stop=True)
                    nc.tensor.matmul(psD[:], _mm(ones_row[:]), _mm(t_qub[:, 512:]),
                                     start=False, stop=True)

                    q_sb = sc2.tile([P, NQ], F32, name="q_sb", tag="q_sb")
                    nc.vector.tensor_copy(q_sb[:, :512], psC[:])
                    nc.vector.tensor_copy(q_sb[:, 512:], psD[:])

                    for h in range(HPC):
                        base = h * QK_TOTAL
                        transpose_to(qsT[h][:, sb * P:(sb + 1) * P],
                                     q_sb[:, base:base + QK_STATIC], P, P)
                        qr = sc2.tile([P, QK_ROT], F32, name="qr", tag="qr")
                        rope(sc2, qr, q_sb[:, base + QK_STATIC:base + QK_TOTAL], sb)
                        transpose_to(qrT[h][:, sb * P:(sb + 1) * P], qr[:], P, QK_ROT)

            # ---------- PHASE 3: attention ----------
            with tc.tile_pool(name="attn_pool", bufs=1) as ap, \
                 tc.tile_pool(name="attn_sc", bufs=2) as asc, \
                 tc.tile_pool(name="probp", bufs=2) as probp, \
                 tc.tile_pool(name="pp3", bufs=4, space="PSUM") as pp, \
                 tc.tile_pool(name="pp3c", bufs=1, space="PSUM") as ppc:
                wk_t, wv_t = [], []
                for h in range(HPC):
                    for cc in range(4):
                        tk = ap.tile([P, QK_STATIC], MMDT, name=f"wk{h}_{cc}",
                                     tag=f"wk{h}_{cc}")
                        nc.sync.dma_start(tk[:], wkT[h, cc * P:(cc + 1) * P, :])
                        tv = ap.tile([P, V_DIM], MMDT, name=f"wv{h}_{cc}",
                                     tag=f"wv{h}_{cc}")
                        nc.sync.dma_start(tv[:], wvT[h, cc * P:(cc + 1) * P, :])
                        wk_t.append(tk)
                        wv_t.append(tv)

                for h in range(HPC):
                    keff = asc.tile([P, SEQ], MMDT, name="keff", tag="keff")
                    veff = asc.tile([P, SEQ], F32, name="veff", tag="veff")
                    for tb in range(2):
                        psk = pp.tile([P, 512], F32, name="ps_keff", tag="ps")
                        psv = pp.tile([P, 512], F32, name="ps_veff", tag="ps")
                        for cc in range(4):
                            nc.tensor.matmul(psk[:], _mm(wk_t[h * 4 + cc][:]),
                                             _mm(kvnT[cc][:, tb * 512:(tb + 1) * 512]),
                                             start=(cc == 0), stop=(cc == 3))
                            nc.tensor.matmul(psv[:], _mm(wv_t[h * 4 + cc][:]),
                                             _mm(kvnT[cc][:, tb * 512:(tb + 1) * 512]),
                                             start=(cc == 0), stop=(cc == 3))
                        nc.vector.tensor_copy(keff[:, tb * 512:(tb + 1) * 512], psk[:])
                        nc.vector.tensor_copy(veff[:, tb * 512:(tb + 1) * 512], psv[:])

                    veffT = asc.tile([P, SEQ], MMDT, name="veffT", tag="veffT")
                    for tcn in range(NSB):
                        transpose_to(veffT[:, tcn * P:(tcn + 1) * P],
                                     veff[:, tcn * P:(tcn + 1) * P], P, P)

                    for shalf in range(2):
                        pT = [probp.tile([P, 512], MMDT, name=f"pT{tcn}", tag=f"pT{tcn}")
                              for tcn in range(NSB)]
                        for sb4 in range(4):
                            sb = shalf * 4 + sb4
                            ps0 = pp.tile([P, 512], F32, name="ps_sc0", tag="ps")
                            ps1 = pp.tile([P, 512], F32, name="ps_sc1", tag="ps")
                            for tb, pstb in enumerate((ps0, ps1)):
                                nc.tensor.matmul(pstb[:],
                                                 _mm(qsT[h][:, sb * P:(sb + 1) * P]),
                                                 _mm(keff[:, tb * 512:(tb + 1) * 512]),
                                                 start=True, stop=False)
                                nc.tensor.matmul(pstb[:],
                                                 _mm(qrT[h][:, sb * P:(sb + 1) * P]),
                                                 _mm(krT[:, tb * 512:(tb + 1) * 512]),
                                                 start=False, stop=True)
                            mt = asc.tile([P, SEQ], F32, name="mask_t", tag="mask_t")
                            nc.sync.dma_start(mt[:], mask[sb * P:(sb + 1) * P, :])
                            scs = asc.tile([P, SEQ], F32, name="scores", tag="scores")
                            nc.vector.tensor_tensor(scs[:, :512], ps0[:], mt[:, :512],
                                                    op=mybir.AluOpType.add)
                            nc.vector.tensor_tensor(scs[:, 512:], ps1[:], mt[:, 512:],
                                                    op=mybir.AluOpType.add)
                            mx = scv.tile([P, 1], F32, name="mx", tag="mx")
                            nc.vector.reduce_max(mx[:], scs[:],
                                                 axis=mybir.AxisListType.X)
                            negmax = scv.tile([P, 1], F32, name="negmax", tag="negmax")
                            nc.vector.tensor_scalar(negmax[:], mx[:], -1.0, None,
                                                    op0=mybir.AluOpType.mult)
                            probs = asc.tile([P, SEQ], F32, name="probs", tag="probs")
                            rowsum = scv.tile([P, 1], F32, name="rowsum", tag="rowsum")
                            nc.scalar.activation(probs[:], scs[:],
                                                 mybir.ActivationFunctionType.Exp,
                                                 bias=negmax[:], accum_out=rowsum[:])
                            logsum = scv.tile([P, 1], F32, name="logsum", tag="logsum")
                            nc.scalar.activation(logsum[:], rowsum[:],
                                                 mybir.ActivationFunctionType.Ln)
                            bias2 = scv.tile([P, 1], F32, name="bias2", tag="bias2")
                            nc.vector.tensor_tensor(bias2[:], negmax[:], logsum[:],
                                                    op=mybir.AluOpType.subtract)
                            nc.scalar.activation(probs[:], scs[:],
                                                 mybir.ActivationFunctionType.Exp,
                                                 bias=bias2[:])
                            for tcn in range(NSB):
                                transpose_to(pT[tcn][:, sb4 * P:(sb4 + 1) * P],
                                             probs[:, tcn * P:(tcn + 1) * P], P, P)

                        psx = ppc.tile([P, 512], F32, name="ps_ctx", tag="ps_ctx")
                        for tcn in range(NSB):
                            nc.tensor.matmul(psx[:],
                                             _mm(veffT[:, tcn * P:(tcn + 1) * P]),
                                             _mm(pT[tcn][:]),
                                             start=(tcn == 0), stop=(tcn == NSB - 1))
                        nc.vector.tensor_copy(
                            ctxT[h][:, shalf * 512:(shalf + 1) * 512], psx[:])

            # ---------- PHASE 4: output projection (partial) ----------
            with tc.tile_pool(name="wo_pool", bufs=1) as wp4, \
                 tc.tile_pool(name="sc4", bufs=2) as sc4, \
                 tc.tile_pool(name="pp4", bufs=4, space="PSUM") as pp:
                wo = []
                for h in range(HPC):
                    t = wp4.tile([P, DIM], MMDT, name=f"wo{h}", tag=f"wo{h}")
                    nc.sync.dma_start(t[:], woutT[h * P:(h + 1) * P, :])
                    wo.append(t)

                for sb in range(NSB):
                    for nb in range(4):
                        pso = pp.tile([P, 512], F32, name="ps_out", tag="ps")
                        for h in range(HPC):
                            nc.tensor.matmul(pso[:],
                                             _mm(ctxT[h][:, sb * P:(sb + 1) * P]),
                                             _mm(wo[h][:, nb * 512:(nb + 1) * 512]),
                                             start=(h == 0), stop=False)
                        nc.tensor.matmul(pso[:], _mm(ones_row[:]),
                                         _mm(t_outb[:, nb * 512:(nb + 1) * 512]),
                                         start=False, stop=True)
                        ot = sc4.tile([P, 512], F32, name="ot", tag="ot")
                        nc.vector.tensor_copy(ot[:], pso[:])
                        nc.sync.dma_start(out_p[sb * P:(sb + 1) * P,
                                                nb * 512:(nb + 1) * 512], ot[:])

    nc.compile()
    return nc


def prep_core_inputs(x, mask, q_down_w, q_down_b, q_norm_scale, q_up_w, q_up_b,
                     kv_down_w, kv_down_b, kv_norm_scale, kv_up_w, out_w, out_b):
    """Host-side shard/transpose prep. Returns list of 8 in_maps."""
    f = np.float32
    inv = f(1.0 / np.sqrt(QK_TOTAL))

    wqdT = np.ascontiguousarray(q_down_w.T, dtype=f)
    wkvdT = np.ascontiguousarray(kv_down_w.T, dtype=f)
    qdb = q_down_b.reshape(1, -1).astype(f)
    kvdb = kv_down_b.reshape(1, -1).astype(f)

    q_up_eff = (q_up_w.astype(f) * q_norm_scale[None, :].astype(f)) * inv
    qub_eff = (q_up_b.astype(f) * inv).reshape(H, QK_TOTAL)

    wk_all = kv_up_w[:H * QK_STATIC].reshape(H, QK_STATIC, KV_RANK).astype(f)
    wv_all = kv_up_w[-H * V_DIM:].reshape(H, V_DIM, KV_RANK).astype(f)
    kvs = kv_norm_scale.astype(f)

    # rope tables (positions 0..SEQ-1)
    invf = 1.0 / (10000.0 ** (np.arange(0, QK_ROT, 2, dtype=np.float64) / QK_ROT))
    freqs = np.arange(SEQ, dtype=np.float64)[:, None] * invf[None, :]
    cosf = np.concatenate([np.cos(freqs), np.cos(freqs)], axis=-1).astype(f)
    sinf = np.concatenate([np.sin(freqs), np.sin(freqs)], axis=-1).astype(f)
    sinhr = sinf.copy()
    sinhr[:, :QK_ROT // 2] *= -1.0  # pre-negated lower half

    in_maps = []
    for c in range(N_CORES):
        b, g = c // 4, c % 4
        hs = slice(g * HPC, (g + 1) * HPC)
        wqupT = np.ascontiguousarray(
            q_up_eff.reshape(H, QK_TOTAL, Q_RANK)[hs].reshape(HPC * QK_TOTAL, Q_RANK).T,
            dtype=f)
        qub = qub_eff[hs].reshape(1, HPC * QK_TOTAL)
        wkT = np.ascontiguousarray(
            (wk_all[hs] * kvs[None, None, :]).transpose(0, 2, 1), dtype=f)
        wvT = np.ascontiguousarray(
            (wv_all[hs] * kvs[None, None, :]).transpose(0, 2, 1), dtype=f)
        woutT = np.ascontiguousarray(
            out_w[:, g * HPC * V_DIM:(g + 1) * HPC * V_DIM].T, dtype=f)
        outb = (out_b if g == 0 else np.zeros_like(out_b)).reshape(1, -1).astype(f)
        in_maps.append({
            "xT": np.ascontiguousarray(x[b].T, dtype=f),
            "mask": np.ascontiguousarray(mask[b], dtype=f),
            "wqdT": wqdT, "qdb": qdb,
            "wkvdT": wkvdT, "kvdb": kvdb,
            "wqupT": wqupT, "qub": np.ascontiguousarray(qub),
            "wkT": wkT, "wvT": wvT,
            "woutT": woutT, "outb": outb,
            "cosf": cosf, "sinhr": sinhr,
            "ones_in": np.ones((1, P), dtype=f),
        })
    return in_maps


_NC_CACHE = None


def kernel(**inputs):
    global _NC_CACHE
    x = np.asarray(inputs["x"], dtype=np.float32)
    args = {k: np.asarray(v) for k, v in inputs.items()
            if k not in ("x", "start_pos")}
    in_maps = prep_core_inputs(x=x, **{k: args[k] for k in (
        "mask", "q_down_w", "q_down_b", "q_norm_scale", "q_up_w", "q_up_b",
        "kv_down_w", "kv_down_b", "kv_norm_scale", "kv_up_w", "out_w", "out_b")})
    if _NC_CACHE is None:
        _NC_CACHE = build_kernel()
    res = run_bass_kernel_spmd(_NC_CACHE, in_maps, list(range(N_CORES))).results
    out = np.zeros((BS, SEQ, DIM), dtype=np.float32)
    for c in range(N_CORES):
        out[c // 4] += res[c]["out_p"]
    return out



# revision 14
# speedup vs baseline: 1.5299x; 1.5299x over previous
"""MLA (multi-head latent attention) prefill kernel for 8 TRN2 NeuronCores.

Sharding: 4 head-groups x 2 batches. Core c: batch = c // 4, head-group g = c % 4
(4 heads each). Each core computes its batch's down-projections + RMSNorm,
its 4 heads' q_up / attention / ctx, and a partial output projection
(out_w column slice). Host sums the 4 partials per batch (TP unshard).

v3: bf16 operands throughout (FWL, half DMA). Attention computes scores
TRANSPOSED [t, s] directly (both operand orientations already exist), so
the probability matrix never needs transposing: probsT = exp(scoresT +
maskT) raw (scores are O(10), no max subtraction needed), row sums come
from ones-matmuls, and softmax normalization is folded into the ctx PSUM
evacuation via a rank-1 broadcast of 1/rowsum. veffT is computed directly
in transposed form. DVE-based RMSNorm stats; single ACT table swap.
"""

import sys
import os

for _p in ("/opt/trn_rl_repo", "/root/.axon_site/_ro/trn_rl_repo"):
    if os.path.isdir(_p) and _p not in sys.path:
        sys.path.insert(0, _p)

import numpy as np

import concourse.bass as bass
import concourse.bacc as bacc
import concourse.tile as tile
import concourse.mybir as mybir
from concourse.bass_utils import run_bass_kernel_spmd
from concourse.masks import make_identity

F32 = mybir.dt.float32
BF16 = mybir.dt.bfloat16
AF = mybir.ActivationFunctionType
ALU = mybir.AluOpType

DIM, H, Q_RANK, KV_RANK = 2048, 16, 768, 512
QK_STATIC, QK_ROT, V_DIM = 128, 64, 128
QK_TOTAL = QK_STATIC + QK_ROT
BS, SEQ = 2, 1024
HPC = 4          # heads per core
N_CORES = 8
P = 128
NSB = SEQ // P   # 8 s-blocks
NKD = DIM // P   # 16 d-chunks
DCAT = Q_RANK + KV_RANK + QK_ROT   # 1344 fused down-proj output cols
NQU = HPC * QK_TOTAL               # 768 q_up cols for this core


def build_kernel():
    nc = bacc.Bacc("TRN2", target_bir_lowering=False, debug=False)

    def din(name, shape, dt=BF16):
        return nc.dram_tensor(name, list(shape), dt, kind="ExternalInput")

    xt = din("xt", (P, NSB * NKD * P))           # x^T tiles per (sb, k)
    wd = din("wd", (P, NKD * DCAT))              # fused down-proj weights
    bcat = din("bcat", (1, DCAT))                # fused down-proj bias row
    wqu = din("wqu", (P, 6 * NQU))               # q_up weights (6 r-chunks)
    qub = din("qub", (1, NQU))                   # q_up bias row
    wk = din("wk", (P, HPC * 4 * P))             # absorbed key weights
    wv = din("wv", (P, HPC * 4 * P))             # absorbed value weights
    wo = din("wo", (P, HPC * DIM))               # out-proj slice
    ob = din("ob", (1, DIM))                     # out bias row (core g==0)
    maskT = din("maskT", (P, NSB * SEQ))         # mask^T tiles per t-block
    cs4 = din("cs4", (P, NSB * 512))             # rope tables per sb (x4 heads)

    out_p = nc.dram_tensor("out_p", [SEQ, DIM], F32, kind="ExternalOutput")

    with tile.TileContext(nc) as tc:
        import contextlib
        ctx = contextlib.ExitStack()
        with ctx:
            const = ctx.enter_context(tc.tile_pool(name="const", bufs=1))
            pers = ctx.enter_context(tc.tile_pool(name="pers", bufs=1))
            scv = ctx.enter_context(tc.tile_pool(name="scv", bufs=4))

            ident = const.tile([P, P], BF16, tag="ident")
            make_identity(nc, ident[:])
            ones1 = const.tile([1, P], BF16, tag="ones1")
            nc.gpsimd.memset(ones1[:], 1.0)
            onesc = const.tile([P, 1], BF16, tag="onesc")
            nc.gpsimd.memset(onesc[:], 1.0)

            t_cs4 = const.tile([P, NSB * 512], BF16, tag="cs4")
            nc.sync.dma_start(t_cs4[:], cs4[:])
            t_bcat = const.tile([1, DCAT], BF16, tag="bcat")
            nc.sync.dma_start(t_bcat[:], bcat[:])
            t_qub = const.tile([1, NQU], BF16, tag="qub")
            nc.sync.dma_start(t_qub[:], qub[:])
            t_ob = const.tile([1, DIM], BF16, tag="ob")
            nc.sync.dma_start(t_ob[:], ob[:])

            # persistent activations
            kvnT = pers.tile([P, 4 * SEQ], BF16, tag="kvnT")        # 4 c-chunks
            krT = pers.tile([64, SEQ], BF16, tag="krT")
            qnT_all = pers.tile([P, 6 * SEQ], BF16, tag="qnT_all")  # 6 r-chunks
            qsT = pers.tile([P, HPC * SEQ], BF16, tag="qsT")        # per head
            qrT = [pers.tile([64, SEQ], BF16, name=f"qrT{h}", tag=f"qrT{h}")
                   for h in range(HPC)]
            ctxT = [pers.tile([P, SEQ], BF16, name=f"ctxT{h}", tag=f"ctxT{h}")
                    for h in range(HPC)]

            # broadcast bias tiles (bias value replicated down partitions)
            bias_bc = pers.tile([P, DCAT], F32, tag="bias_bc")
            qub_bc = pers.tile([P, NQU], F32, tag="qub_bc")
            ob_bc = pers.tile([P, DIM], F32, tag="ob_bc")
            with tc.tile_pool(name="ppbc", bufs=2, space="PSUM") as ppbc:
                for dst, src, w in ((bias_bc, t_bcat, DCAT), (qub_bc, t_qub, NQU),
                                    (ob_bc, t_ob, DIM)):
                    for n0 in range(0, w, 512):
                        n1 = min(n0 + 512, w)
                        psb = ppbc.tile([P, 512], F32, name="ps_bc", tag="ps_bc")
                        nc.tensor.matmul(psb[:, 0:n1 - n0], ones1[:], src[:, n0:n1],
                                         start=True, stop=True)
                        nc.vector.tensor_copy(dst[:, n0:n1], psb[:, 0:n1 - n0])

            def rope(dst_ap, src_ap, sb, width):
                # dst/src: [P, 2*width]; tables: cos_lo|cos_hi|sinhr_lo|sinhr_hi
                cb = sb * 512
                c_lo = t_cs4[:, cb:cb + width]
                c_hi = t_cs4[:, cb + 128:cb + 128 + width]
                s_lo = t_cs4[:, cb + 256:cb + 256 + width]
                s_hi = t_cs4[:, cb + 384:cb + 384 + width]
                m1 = scv.tile([P, 128], BF16, name="rp1", tag="rp1")
                m2 = scv.tile([P, 128], BF16, name="rp2", tag="rp2")
                lo, hi = src_ap[:, 0:width], src_ap[:, width:2 * width]
                nc.vector.tensor_tensor(m1[:, 0:width], lo, c_lo, op=ALU.mult)
                nc.vector.tensor_tensor(m2[:, 0:width], hi, s_lo, op=ALU.mult)
                nc.vector.tensor_tensor(dst_ap[:, 0:width], m1[:, 0:width],
                                        m2[:, 0:width], op=ALU.add)
                nc.vector.tensor_tensor(m1[:, 0:width], hi, c_hi, op=ALU.mult)
                nc.vector.tensor_tensor(m2[:, 0:width], lo, s_hi, op=ALU.mult)
                nc.vector.tensor_tensor(dst_ap[:, width:2 * width], m1[:, 0:width],
                                        m2[:, 0:width], op=ALU.add)

            # ---------- PHASES D + Q: down-proj, norm, q_up ----------
            with tc.tile_pool(name="wdq", bufs=1) as wdq, \
                 tc.tile_pool(name="ppt", bufs=2, space="PSUM") as ppt:

                def transpose_to(dst_ap, src_ap, rows, cols):
                    pst = ppt.tile([P, P], BF16, name="tr", tag="tr")
                    nc.tensor.transpose(pst[0:cols, 0:rows], src_ap,
                                        ident[:rows, :rows])
                    nc.vector.tensor_copy(dst_ap, pst[0:cols, 0:rows])

                wd_t = wdq.tile([P, NKD * DCAT], BF16, tag="wd_t")
                for q in range(4):
                    nc.sync.dma_start(wd_t[:, q * 4 * DCAT:(q + 1) * 4 * DCAT],
                                      wd[:, q * 4 * DCAT:(q + 1) * 4 * DCAT])
                wqu_t = wdq.tile([P, 6 * NQU], BF16, tag="wqu_t")
                nc.sync.dma_start(wqu_t[:], wqu[:])

                # -- phase D: fused down-proj + RMSNorm per s-block --
                with tc.tile_pool(name="xs", bufs=2) as xs_pool, \
                     tc.tile_pool(name="dqs", bufs=2) as dqs, \
                     tc.tile_pool(name="ppdq", bufs=2, space="PSUM") as ppdq:
                    for sb in range(NSB):
                        x_sb = xs_pool.tile([P, NKD * P], BF16, name="x_sb",
                                            tag="x_sb")
                        nc.sync.dma_start(x_sb[:],
                                          xt[:, sb * NKD * P:(sb + 1) * NKD * P])

                        ps = ppdq.tile([P, DCAT], F32, name="psd", tag="psd")
                        for k in range(NKD):
                            xk = x_sb[:, k * P:(k + 1) * P]
                            wb = k * DCAT
                            nc.tensor.matmul(ps[:, 0:512], xk, wd_t[:, wb:wb + 512],
                                             start=(k == 0), stop=(k == NKD - 1))
                            nc.tensor.matmul(ps[:, 512:1024], xk,
                                             wd_t[:, wb + 512:wb + 1024],
                                             start=(k == 0), stop=(k == NKD - 1))
                            nc.tensor.matmul(ps[:, 1024:1344], xk,
                                             wd_t[:, wb + 1024:wb + 1344],
                                             start=(k == 0), stop=(k == NKD - 1))

                        # bias add into fp32 scratch (also the norm input)
                        tmp = dqs.tile([P, DCAT], F32, name="tmp", tag="tmp")
                        nc.vector.tensor_tensor(tmp[:], ps[:], bias_bc[:],
                                                op=ALU.add)

                        # RMSNorm stats (DVE square+reduce, ACT sqrt)
                        sqf = dqs.tile([P, Q_RANK + KV_RANK], F32,
                                       name="sqf", tag="sqf")
                        ssq_q = scv.tile([P, 1], F32, name="ssq_q", tag="ssq_q")
                        ssq_kv = scv.tile([P, 1], F32, name="ssq_kv", tag="ssq_kv")
                        nc.vector.tensor_tensor(sqf[:], tmp[:, 0:Q_RANK + KV_RANK],
                                                tmp[:, 0:Q_RANK + KV_RANK],
                                                op=ALU.mult)
                        nc.vector.tensor_reduce(ssq_q[:], sqf[:, 0:Q_RANK],
                                                axis=mybir.AxisListType.X,
                                                op=ALU.add)
                        nc.vector.tensor_reduce(ssq_kv[:], sqf[:, Q_RANK:],
                                                axis=mybir.AxisListType.X,
                                                op=ALU.add)

                        def rstd_of(ssq, n, nm):
                            ms = scv.tile([P, 1], F32, name=nm + "m", tag=nm + "m")
                            nc.vector.tensor_scalar(ms[:], ssq[:], 1.0 / n, 1e-6,
                                                    op0=ALU.mult, op1=ALU.add)
                            ri = scv.tile([P, 1], F32, name=nm + "i", tag=nm + "i")
                            nc.vector.reciprocal(ri[:], ms[:])
                            rs = scv.tile([P, 1], F32, name=nm + "s", tag=nm + "s")
                            nc.scalar.sqrt(rs[:], ri[:])
                            return rs

                        rstd_q = rstd_of(ssq_q, Q_RANK, "rq")
                        rstd_kv = rstd_of(ssq_kv, KV_RANK, "rk")

                        # normalized q latent (bf16) -> 6 transposed chunks
                        qn = dqs.tile([P, Q_RANK], BF16, name="qn", tag="qn")
                        nc.vector.tensor_scalar(qn[:], tmp[:, 0:Q_RANK], rstd_q[:],
                                                None, op0=ALU.mult)
                        for rc in range(6):
                            transpose_to(
                                qnT_all[:, rc * SEQ + sb * P:rc * SEQ + (sb + 1) * P],
                                qn[:, rc * P:(rc + 1) * P], P, P)

                        # normalized kv latent -> kvnT chunks
                        kvn = dqs.tile([P, KV_RANK], BF16, name="kvn", tag="kvn")
                        nc.vector.tensor_scalar(kvn[:],
                                                tmp[:, Q_RANK:Q_RANK + KV_RANK],
                                                rstd_kv[:], None, op0=ALU.mult)
                        for cc in range(4):
                            transpose_to(
                                kvnT[:, cc * SEQ + sb * P:cc * SEQ + (sb + 1) * P],
                                kvn[:, cc * P:(cc + 1) * P], P, P)

                        # k_rot: rope then transpose
                        krp = dqs.tile([P, QK_ROT], BF16, name="krp", tag="krp")
                        rope(krp[:], tmp[:, Q_RANK + KV_RANK:DCAT], sb, 32)
                        transpose_to(krT[:, sb * P:(sb + 1) * P], krp[:], P, QK_ROT)

                # -- phase Q: q_up + rope + transposes per s-block --
                with tc.tile_pool(name="qs2", bufs=2) as qs2, \
                     tc.tile_pool(name="ppqu", bufs=2, space="PSUM") as ppqu:
                    for sb in range(NSB):
                        psq = ppqu.tile([P, NQU], F32, name="psq", tag="psq")
                        for rc in range(6):
                            qk = qnT_all[:, rc * SEQ + sb * P:rc * SEQ + (sb + 1) * P]
                            nc.tensor.matmul(psq[:, 0:512], qk,
                                             wqu_t[:, rc * NQU:rc * NQU + 512],
                                             start=(rc == 0), stop=(rc == 5))
                            nc.tensor.matmul(psq[:, 512:NQU], qk,
                                             wqu_t[:, rc * NQU + 512:(rc + 1) * NQU],
                                             start=(rc == 0), stop=(rc == 5))

                        q_sb = qs2.tile([P, NQU], BF16, name="q_sb", tag="q_sb")
                        nc.vector.tensor_tensor(q_sb[:], psq[:], qub_bc[:],
                                                op=ALU.add)

                        # static parts -> qsT per head
                        for h in range(HPC):
                            transpose_to(
                                qsT[:, h * SEQ + sb * P:h * SEQ + (sb + 1) * P],
                                q_sb[:, h * P:(h + 1) * P], P, P)
                        # rot part: cols [512:640]=lo x4 heads, [640:768]=hi x4
                        qrot = qs2.tile([P, 256], BF16, name="qrot", tag="qrot")
                        rope(qrot[:], q_sb[:, 512:NQU], sb, 128)
                        for h in range(HPC):
                            transpose_to(qrT[h][0:32, sb * P:(sb + 1) * P],
                                         qrot[:, h * 32:(h + 1) * 32], P, 32)
                            transpose_to(qrT[h][32:64, sb * P:(sb + 1) * P],
                                         qrot[:, 128 + h * 32:128 + (h + 1) * 32],
                                         P, 32)

            # ---------- PHASE A0: effective K / transposed V per head ----------
            kvf = ctx.enter_context(tc.tile_pool(name="kvf", bufs=1))
            keff = kvf.tile([P, HPC * SEQ], BF16, tag="keff")
            veffT = kvf.tile([P, HPC * SEQ], BF16, tag="veffT")
            with tc.tile_pool(name="wkv", bufs=1) as wkv, \
                 tc.tile_pool(name="ppa0", bufs=2, space="PSUM") as ppa0, \
                 tc.tile_pool(name="ppav", bufs=4, space="PSUM") as ppav:
                wk_t = wkv.tile([P, HPC * 4 * P], BF16, tag="wk_t")
                nc.sync.dma_start(wk_t[:], wk[:])
                wv_t = wkv.tile([P, HPC * 4 * P], BF16, tag="wv_t")
                nc.sync.dma_start(wv_t[:], wv[:])

                for h in range(HPC):
                    # keff[qk, t] = wk_h^T kvn^T  (accumulate c-chunks)
                    for th in range(2):
                        psk = ppa0.tile([P, 512], F32, name="psk", tag="psk")
                        for cc in range(4):
                            ks = kvnT[:, cc * SEQ + th * 512:cc * SEQ + (th + 1) * 512]
                            nc.tensor.matmul(
                                psk[:],
                                wk_t[:, (h * 4 + cc) * P:(h * 4 + cc + 1) * P],
                                ks, start=(cc == 0), stop=(cc == 3))
                        nc.vector.tensor_copy(
                            keff[:, h * SEQ + th * 512:h * SEQ + (th + 1) * 512],
                            psk[:])
                    # veffT[t, v] = kvn wv_h  (lhsT = kvnT chunk slices)
                    for tb in range(NSB):
                        psv = ppav.tile([P, P], F32, name="psv", tag="psv")
                        for cc in range(4):
                            nc.tensor.matmul(
                                psv[:],
                                kvnT[:, cc * SEQ + tb * P:cc * SEQ + (tb + 1) * P],
                                wv_t[:, (h * 4 + cc) * P:(h * 4 + cc + 1) * P],
                                start=(cc == 0), stop=(cc == 3))
                        nc.vector.tensor_copy(
                            veffT[:, h * SEQ + tb * P:h * SEQ + (tb + 1) * P],
                            psv[:])

            # ---------- PHASE A1: transposed scores + softmax + ctx ----------
            mskp = ctx.enter_context(tc.tile_pool(name="mskp", bufs=1))
            maskT_t = mskp.tile([P, NSB * SEQ], BF16, tag="maskT_t")
            nc.sync.dma_start(maskT_t[:], maskT[:])
            with tc.tile_pool(name="att", bufs=2) as att, \
                 tc.tile_pool(name="ppsc", bufs=2, space="PSUM") as ppsc, \
                 tc.tile_pool(name="pprs", bufs=1, space="PSUM") as pprs, \
                 tc.tile_pool(name="ppcx", bufs=2, space="PSUM") as ppcx:
                for h in range(HPC):
                    probsT = att.tile([P, NSB * SEQ], BF16, name="probsT",
                                      tag="probsT")
                    for tb in range(NSB):
                        ps_sc = ppsc.tile([P, SEQ], F32, name="ps_sc", tag="ps_sc")
                        for sh in range(2):
                            nc.tensor.matmul(
                                ps_sc[:, sh * 512:(sh + 1) * 512],
                                keff[:, h * SEQ + tb * P:h * SEQ + (tb + 1) * P],
                                qsT[:, h * SEQ + sh * 512:h * SEQ + (sh + 1) * 512],
                                start=True, stop=False)
                            nc.tensor.matmul(
                                ps_sc[:, sh * 512:(sh + 1) * 512],
                                krT[:, tb * P:(tb + 1) * P],
                                qrT[h][:, sh * 512:(sh + 1) * 512],
                                start=False, stop=True)
                        scsT = att.tile([P, SEQ], F32, name="scsT", tag="scsT")
                        nc.vector.tensor_tensor(scsT[:], ps_sc[:],
                                                maskT_t[:, tb * SEQ:(tb + 1) * SEQ],
                                                op=ALU.add)
                        nc.scalar.activation(probsT[:, tb * SEQ:(tb + 1) * SEQ],
                                             scsT[:], AF.Exp)
                    for sh in range(2):
                        # row sums via ones-matmul over t
                        ps_rs = pprs.tile([1, 512], F32, name="ps_rs", tag="ps_rs")
                        for tb in range(NSB):
                            nc.tensor.matmul(
                                ps_rs[:], onesc[:],
                                probsT[:, tb * SEQ + sh * 512:tb * SEQ + (sh + 1) * 512],
                                start=(tb == 0), stop=(tb == NSB - 1))
                        rs_row = scv.tile([1, 512], F32, name="rs_row", tag="rs_row")
                        nc.vector.tensor_copy(rs_row[:], ps_rs[:])
                        rcp_row = scv.tile([1, 512], BF16, name="rcp_row",
                                           tag="rcp_row")
                        with nc.allow_low_precision(reason="softmax 1/rowsum bf16"):
                            nc.vector.reciprocal(rcp_row[:], rs_row[:])
                        ps_bc2 = pprs.tile([P, 512], F32, name="ps_bc2",
                                           tag="ps_bc2")
                        nc.tensor.matmul(ps_bc2[:], ones1[:], rcp_row[:],
                                         start=True, stop=True)
                        rcp_bc = att.tile([P, 512], BF16, name="rcp_bc",
                                          tag="rcp_bc")
                        nc.vector.tensor_copy(rcp_bc[:], ps_bc2[:])
                        # ctx (unnormalized) then normalize during evacuation
                        ps_ctx = ppcx.tile([P, 512], F32, name="ps_ctx",
                                           tag="ps_ctx")
                        for tb in range(NSB):
                            nc.tensor.matmul(
                                ps_ctx[:],
                                veffT[:, h * SEQ + tb * P:h * SEQ + (tb + 1) * P],
                                probsT[:, tb * SEQ + sh * 512:tb * SEQ + (sh + 1) * 512],
                                start=(tb == 0), stop=(tb == NSB - 1))
                        nc.vector.tensor_tensor(ctxT[h][:, sh * 512:(sh + 1) * 512],
                                                ps_ctx[:], rcp_bc[:], op=ALU.mult)

            # ---------- PHASE O: output projection (partial) ----------
            with tc.tile_pool(name="wop", bufs=1) as wop, \
                 tc.tile_pool(name="ost", bufs=2) as ost, \
                 tc.tile_pool(name="ppo", bufs=4, space="PSUM") as ppo:
                wo_t = wop.tile([P, HPC * DIM], BF16, tag="wo_t")
                nc.sync.dma_start(wo_t[:], wo[:])

                for sb in range(NSB):
                    ostage = ost.tile([P, DIM], F32, name="ostage", tag="ostage")
                    for nb in range(4):
                        pso = ppo.tile([P, 512], F32, name="pso", tag="pso")
                        for h in range(HPC):
                            nc.tensor.matmul(
                                pso[:], ctxT[h][:, sb * P:(sb + 1) * P],
                                wo_t[:, h * DIM + nb * 512:h * DIM + (nb + 1) * 512],
                                start=(h == 0), stop=(h == HPC - 1))
                        nc.vector.tensor_tensor(ostage[:, nb * 512:(nb + 1) * 512],
                                                pso[:],
                                                ob_bc[:, nb * 512:(nb + 1) * 512],
                                                op=ALU.add)
                    nc.sync.dma_start(out_p[sb * P:(sb + 1) * P, :], ostage[:])

    nc.compile()
    return nc


def prep_core_inputs(x, mask, q_down_w, q_down_b, q_norm_scale, q_up_w, q_up_b,
                     kv_down_w, kv_down_b, kv_norm_scale, kv_up_w, out_w, out_b):
    """Host-side shard/pack prep. Returns list of 8 in_maps (bf16 tiles)."""
    import ml_dtypes
    bf = ml_dtypes.bfloat16
    f = np.float32
    inv = f(1.0 / np.sqrt(QK_TOTAL))

    # fused down-proj: rows q_down (768) + kv_down (576) -> [1344, 2048]
    wcat = np.concatenate([np.asarray(q_down_w, f), np.asarray(kv_down_w, f)], 0)
    wd = np.ascontiguousarray(
        wcat.T.reshape(NKD, P, DCAT).transpose(1, 0, 2).reshape(P, NKD * DCAT)
    ).astype(bf)
    bcat = np.concatenate([np.asarray(q_down_b, f), np.asarray(kv_down_b, f)]
                          ).reshape(1, DCAT).astype(bf)

    # q_up with norm scale and 1/sqrt(qk_total) folded in
    q_up_eff = (np.asarray(q_up_w, f) * np.asarray(q_norm_scale, f)[None, :]) * inv
    q_up_eff = q_up_eff.reshape(H, QK_TOTAL, Q_RANK)
    qub_eff = (np.asarray(q_up_b, f) * inv).reshape(H, QK_TOTAL)

    wk_all = np.asarray(kv_up_w, f)[:H * QK_STATIC].reshape(H, QK_STATIC, KV_RANK)
    wv_all = np.asarray(kv_up_w, f)[-H * V_DIM:].reshape(H, V_DIM, KV_RANK)
    kvs = np.asarray(kv_norm_scale, f)

    # rope tables: cos/sin with the "half-rotate" sign pre-applied
    invf = 1.0 / (10000.0 ** (np.arange(0, QK_ROT, 2, dtype=np.float64) / QK_ROT))
    freqs = np.arange(SEQ, dtype=np.float64)[:, None] * invf[None, :]
    cosf = np.concatenate([np.cos(freqs), np.cos(freqs)], -1).astype(f)  # (S, 64)
    sinf = np.concatenate([np.sin(freqs), np.sin(freqs)], -1).astype(f)
    sinhr = sinf.copy()
    sinhr[:, :QK_ROT // 2] *= -1.0
    # per sb: [cos_lo x4 | cos_hi x4 | sinhr_lo x4 | sinhr_hi x4] (each 128 cols)
    cs4 = np.zeros((P, NSB * 512), f)
    for sb in range(NSB):
        rows = slice(sb * P, (sb + 1) * P)
        cs4[:, sb * 512 + 0:sb * 512 + 128] = np.tile(cosf[rows, 0:32], 4)
        cs4[:, sb * 512 + 128:sb * 512 + 256] = np.tile(cosf[rows, 32:64], 4)
        cs4[:, sb * 512 + 256:sb * 512 + 384] = np.tile(sinhr[rows, 0:32], 4)
        cs4[:, sb * 512 + 384:sb * 512 + 512] = np.tile(sinhr[rows, 32:64], 4)
    cs4 = cs4.astype(bf)

    x = np.asarray(x, f)
    mask = np.asarray(mask, f)
    out_b = np.asarray(out_b, f)
    out_w = np.asarray(out_w, f)

    in_maps = []
    for c in range(N_CORES):
        b, g = c // 4, c % 4
        habs = [g * HPC + h for h in range(HPC)]

        # x^T tiles: [p, (sb, k, j)] = x[b, sb*128+j, k*128+p]
        xtb = np.ascontiguousarray(
            x[b].reshape(NSB, P, NKD, P).transpose(3, 0, 2, 1).reshape(P, NSB * NKD * P)
        ).astype(bf)

        # q_up cols: [h0..h3 static (512) | lo x4 (128) | hi x4 (128)]
        wq_cols = np.zeros((Q_RANK, NQU), f)
        qub_cols = np.zeros((1, NQU), f)
        for h, ha in enumerate(habs):
            wq_cols[:, h * P:(h + 1) * P] = q_up_eff[ha, :QK_STATIC].T
            wq_cols[:, 512 + h * 32:512 + (h + 1) * 32] = q_up_eff[ha, QK_STATIC:QK_STATIC + 32].T
            wq_cols[:, 640 + h * 32:640 + (h + 1) * 32] = q_up_eff[ha, QK_STATIC + 32:].T
            qub_cols[0, h * P:(h + 1) * P] = qub_eff[ha, :QK_STATIC]
            qub_cols[0, 512 + h * 32:512 + (h + 1) * 32] = qub_eff[ha, QK_STATIC:QK_STATIC + 32]
            qub_cols[0, 640 + h * 32:640 + (h + 1) * 32] = qub_eff[ha, QK_STATIC + 32:]
        wqu_p = np.ascontiguousarray(
            wq_cols.reshape(6, P, NQU).transpose(1, 0, 2).reshape(P, 6 * NQU)
        ).astype(bf)

        wk_p = np.zeros((P, HPC * 4 * P), f)
        wv_p = np.zeros((P, HPC * 4 * P), f)
        for h, ha in enumerate(habs):
            wkT = (wk_all[ha] * kvs[None, :]).T          # [c=512, qk=128]
            wvT = (wv_all[ha] * kvs[None, :]).T
            for cc in range(4):
                wk_p[:, (h * 4 + cc) * P:(h * 4 + cc + 1) * P] = wkT[cc * P:(cc + 1) * P]
                wv_p[:, (h * 4 + cc) * P:(h * 4 + cc + 1) * P] = wvT[cc * P:(cc + 1) * P]

        wo_p = np.zeros((P, HPC * DIM), f)
        for h, ha in enumerate(habs):
            wo_p[:, h * DIM:(h + 1) * DIM] = out_w[:, ha * V_DIM:(ha + 1) * V_DIM].T
        ob_p = (out_b if g == 0 else np.zeros_like(out_b)).reshape(1, DIM)

        # mask^T tiles: block tb = mask[b][:, tb*128:(tb+1)*128].T  -> [t, s]
        maskTb = np.ascontiguousarray(
            mask[b].T.reshape(NSB, P, SEQ).transpose(1, 0, 2).reshape(P, NSB * SEQ)
        ).astype(bf)

        in_maps.append({
            "xt": xtb,
            "wd": wd, "bcat": bcat,
            "wqu": wqu_p, "qub": qub_cols.astype(bf),
            "wk": wk_p.astype(bf), "wv": wv_p.astype(bf),
            "wo": wo_p.astype(bf), "ob": ob_p.astype(bf),
            "maskT": maskTb, "cs4": cs4,
        })
    return in_maps


_NC_CACHE = None


def kernel(**inputs):
    global _NC_CACHE
    x = np.asarray(inputs["x"], dtype=np.float32)
    args = {k: np.asarray(v) for k, v in inputs.items()
            if k not in ("x", "start_pos")}
    in_maps = prep_core_inputs(x=x, **{k: args[k] for k in (
        "mask", "q_down_w", "q_down_b", "q_norm_scale", "q_up_w", "q_up_b",
        "kv_down_w", "kv_down_b", "kv_norm_scale", "kv_up_w", "out_w", "out_b")})
    if _NC_CACHE is None:
        _NC_CACHE = build_kernel()
    res = run_bass_kernel_spmd(_NC_CACHE, in_maps, list(range(N_CORES))).results
    out = np.zeros((BS, SEQ, DIM), dtype=np.float32)
    for c in range(N_CORES):
        out[c // 4] += res[c]["out_p"]
    return out


# revision 16
# speedup vs baseline: 1.8940x; 1.2380x over previous
"""MLA (multi-head latent attention) prefill kernel for 8 TRN2 NeuronCores.

Sharding: 4 head-groups x 2 batches. Core c: batch = c // 4, head-group g = c % 4
(4 heads each). Each core computes its batch's down-projections + RMSNorm,
its 4 heads' q_up / attention / ctx, and a partial output projection
(out_w column slice). Host sums the 4 partials per batch (TP unshard).

v3: bf16 operands throughout (FWL, half DMA). Attention computes scores
TRANSPOSED [t, s] directly (both operand orientations already exist), so
the probability matrix never needs transposing: probsT = exp(scoresT +
maskT) raw (scores are O(10), no max subtraction needed), row sums come
from ones-matmuls, and softmax normalization is folded into the ctx PSUM
evacuation via a rank-1 broadcast of 1/rowsum. veffT is computed directly
in transposed form. DVE-based RMSNorm stats; single ACT table swap.
"""

import sys
import os

for _p in ("/opt/trn_rl_repo", "/root/.axon_site/_ro/trn_rl_repo"):
    if os.path.isdir(_p) and _p not in sys.path:
        sys.path.insert(0, _p)

import numpy as np

import concourse.bass as bass
import concourse.bacc as bacc
import concourse.tile as tile
import concourse.mybir as mybir
from concourse.bass_utils import run_bass_kernel_spmd
from concourse.masks import make_identity

F32 = mybir.dt.float32
BF16 = mybir.dt.bfloat16
AF = mybir.ActivationFunctionType
ALU = mybir.AluOpType

DIM, H, Q_RANK, KV_RANK = 2048, 16, 768, 512
QK_STATIC, QK_ROT, V_DIM = 128, 64, 128
QK_TOTAL = QK_STATIC + QK_ROT
BS, SEQ = 2, 1024
HPC = 4          # heads per core
N_CORES = 8
P = 128
NSB = SEQ // P   # 8 s-blocks
NKD = DIM // P   # 16 d-chunks
DCAT = Q_RANK + KV_RANK + QK_ROT   # 1344 fused down-proj output cols
NQU = HPC * QK_TOTAL               # 768 q_up cols for this core


def build_kernel():
    nc = bacc.Bacc("TRN2", target_bir_lowering=False, debug=False)

    def din(name, shape, dt=BF16):
        return nc.dram_tensor(name, list(shape), dt, kind="ExternalInput")

    xt = din("xt", (P, NSB * NKD * P))           # x^T tiles per (sb, k)
    wd = din("wd", (P, NKD * DCAT))              # fused down-proj weights
    bcat = din("bcat", (1, DCAT))                # fused down-proj bias row
    wqu = din("wqu", (P, 6 * NQU))               # q_up weights (6 r-chunks)
    qub = din("qub", (1, NQU))                   # q_up bias row
    wk = din("wk", (P, HPC * 4 * P))             # absorbed key weights
    wv = din("wv", (P, HPC * 4 * P))             # absorbed value weights
    wo = din("wo", (P, HPC * DIM))               # out-proj slice
    ob = din("ob", (1, DIM))                     # out bias row (core g==0)
    maskT = din("maskT", (P, NSB * SEQ))         # mask^T tiles per t-block
    cs4 = din("cs4", (P, NSB * 512))             # rope tables per sb (x4 heads)

    out_p = nc.dram_tensor("out_p", [SEQ, DIM], F32, kind="ExternalOutput")

    with tile.TileContext(nc) as tc:
        import contextlib
        ctx = contextlib.ExitStack()
        with ctx:
            const = ctx.enter_context(tc.tile_pool(name="const", bufs=1))
            pers = ctx.enter_context(tc.tile_pool(name="pers", bufs=1))
            scv = ctx.enter_context(tc.tile_pool(name="scv", bufs=4))

            ident = const.tile([P, P], BF16, tag="ident")
            make_identity(nc, ident[:])
            ones1 = const.tile([1, P], BF16, tag="ones1")
            nc.gpsimd.memset(ones1[:], 1.0)
            onesc = const.tile([P, 1], BF16, tag="onesc")
            nc.gpsimd.memset(onesc[:], 1.0)

            t_cs4 = const.tile([P, NSB * 512], BF16, tag="cs4")
            nc.sync.dma_start(t_cs4[:], cs4[:])
            t_bcat = const.tile([1, DCAT], BF16, tag="bcat")
            nc.sync.dma_start(t_bcat[:], bcat[:])
            t_qub = const.tile([1, NQU], BF16, tag="qub")
            nc.sync.dma_start(t_qub[:], qub[:])
            t_ob = const.tile([1, DIM], BF16, tag="ob")
            nc.sync.dma_start(t_ob[:], ob[:])

            # persistent activations
            kvnT = pers.tile([P, 4 * SEQ], BF16, tag="kvnT")        # 4 c-chunks
            krT = pers.tile([64, SEQ], BF16, tag="krT")
            qnT_all = pers.tile([P, 6 * SEQ], BF16, tag="qnT_all")  # 6 r-chunks
            qsT = pers.tile([P, HPC * SEQ], BF16, tag="qsT")        # per head
            qrT = [pers.tile([64, SEQ], BF16, name=f"qrT{h}", tag=f"qrT{h}")
                   for h in range(HPC)]
            ctxT = [pers.tile([P, SEQ], BF16, name=f"ctxT{h}", tag=f"ctxT{h}")
                    for h in range(HPC)]

            # broadcast bias tiles (bias value replicated down partitions)
            bias_bc = pers.tile([P, DCAT], F32, tag="bias_bc")
            qub_bc = pers.tile([P, NQU], F32, tag="qub_bc")
            ob_bc = pers.tile([P, DIM], F32, tag="ob_bc")
            with tc.tile_pool(name="ppbc", bufs=2, space="PSUM") as ppbc:
                for dst, src, w in ((bias_bc, t_bcat, DCAT), (qub_bc, t_qub, NQU),
                                    (ob_bc, t_ob, DIM)):
                    for n0 in range(0, w, 512):
                        n1 = min(n0 + 512, w)
                        psb = ppbc.tile([P, 512], F32, name="ps_bc", tag="ps_bc")
                        nc.tensor.matmul(psb[:, 0:n1 - n0], ones1[:], src[:, n0:n1],
                                         start=True, stop=True)
                        nc.vector.tensor_copy(dst[:, n0:n1], psb[:, 0:n1 - n0])

            def rope(dst_ap, src_ap, sb, width):
                # dst/src: [P, 2*width]; tables: cos_lo|cos_hi|sinhr_lo|sinhr_hi
                cb = sb * 512
                c_lo = t_cs4[:, cb:cb + width]
                c_hi = t_cs4[:, cb + 128:cb + 128 + width]
                s_lo = t_cs4[:, cb + 256:cb + 256 + width]
                s_hi = t_cs4[:, cb + 384:cb + 384 + width]
                m1 = scv.tile([P, 128], BF16, name="rp1", tag="rp1")
                m2 = scv.tile([P, 128], BF16, name="rp2", tag="rp2")
                lo, hi = src_ap[:, 0:width], src_ap[:, width:2 * width]
                nc.vector.tensor_tensor(m1[:, 0:width], lo, c_lo, op=ALU.mult)
                nc.vector.tensor_tensor(m2[:, 0:width], hi, s_lo, op=ALU.mult)
                nc.vector.tensor_tensor(dst_ap[:, 0:width], m1[:, 0:width],
                                        m2[:, 0:width], op=ALU.add)
                nc.vector.tensor_tensor(m1[:, 0:width], hi, c_hi, op=ALU.mult)
                nc.vector.tensor_tensor(m2[:, 0:width], lo, s_hi, op=ALU.mult)
                nc.vector.tensor_tensor(dst_ap[:, width:2 * width], m1[:, 0:width],
                                        m2[:, 0:width], op=ALU.add)

            # ---------- PHASES D + Q: down-proj, norm, q_up ----------
            with tc.tile_pool(name="wdq", bufs=1) as wdq, \
                 tc.tile_pool(name="ppt", bufs=2, space="PSUM") as ppt:

                def transpose_to(dst_ap, src_ap, rows, cols):
                    pst = ppt.tile([P, P], BF16, name="tr", tag="tr")
                    nc.tensor.transpose(pst[0:cols, 0:rows], src_ap,
                                        ident[:rows, :rows])
                    nc.vector.tensor_copy(dst_ap, pst[0:cols, 0:rows])

                wd_t = wdq.tile([P, NKD * DCAT], BF16, tag="wd_t")
                for q in range(4):
                    nc.sync.dma_start(wd_t[:, q * 4 * DCAT:(q + 1) * 4 * DCAT],
                                      wd[:, q * 4 * DCAT:(q + 1) * 4 * DCAT])
                wqu_t = wdq.tile([P, 6 * NQU], BF16, tag="wqu_t")
                nc.sync.dma_start(wqu_t[:], wqu[:])

                # -- phase D: fused down-proj + RMSNorm per s-block --
                with tc.tile_pool(name="xs", bufs=2) as xs_pool, \
                     tc.tile_pool(name="dqs", bufs=2) as dqs, \
                     tc.tile_pool(name="ppdq", bufs=2, space="PSUM") as ppdq:
                    for sb in range(NSB):
                        x_sb = xs_pool.tile([P, NKD * P], BF16, name="x_sb",
                                            tag="x_sb")
                        nc.sync.dma_start(x_sb[:],
                                          xt[:, sb * NKD * P:(sb + 1) * NKD * P])

                        ps = ppdq.tile([P, DCAT], F32, name="psd", tag="psd")
                        for k in range(NKD):
                            xk = x_sb[:, k * P:(k + 1) * P]
                            wb = k * DCAT
                            nc.tensor.matmul(ps[:, 0:512], xk, wd_t[:, wb:wb + 512],
                                             start=(k == 0), stop=(k == NKD - 1))
                            nc.tensor.matmul(ps[:, 512:1024], xk,
                                             wd_t[:, wb + 512:wb + 1024],
                                             start=(k == 0), stop=(k == NKD - 1))
                            nc.tensor.matmul(ps[:, 1024:1344], xk,
                                             wd_t[:, wb + 1024:wb + 1344],
                                             start=(k == 0), stop=(k == NKD - 1))

                        # bias add into fp32 scratch (also the norm input)
                        tmp = dqs.tile([P, DCAT], F32, name="tmp", tag="tmp")
                        nc.vector.tensor_tensor(tmp[:], ps[:], bias_bc[:],
                                                op=ALU.add)

                        # RMSNorm stats (DVE square+reduce, ACT sqrt)
                        sqf = dqs.tile([P, Q_RANK + KV_RANK], F32,
                                       name="sqf", tag="sqf")
                        ssq_q = scv.tile([P, 1], F32, name="ssq_q", tag="ssq_q")
                        ssq_kv = scv.tile([P, 1], F32, name="ssq_kv", tag="ssq_kv")
                        nc.vector.tensor_tensor(sqf[:], tmp[:, 0:Q_RANK + KV_RANK],
                                                tmp[:, 0:Q_RANK + KV_RANK],
                                                op=ALU.mult)
                        nc.vector.tensor_reduce(ssq_q[:], sqf[:, 0:Q_RANK],
                                                axis=mybir.AxisListType.X,
                                                op=ALU.add)
                        nc.vector.tensor_reduce(ssq_kv[:], sqf[:, Q_RANK:],
                                                axis=mybir.AxisListType.X,
                                                op=ALU.add)

                        def rstd_of(ssq, n, nm):
                            ms = scv.tile([P, 1], F32, name=nm + "m", tag=nm + "m")
                            nc.vector.tensor_scalar(ms[:], ssq[:], 1.0 / n, 1e-6,
                                                    op0=ALU.mult, op1=ALU.add)
                            ri = scv.tile([P, 1], F32, name=nm + "i", tag=nm + "i")
                            nc.vector.reciprocal(ri[:], ms[:])
                            rs = scv.tile([P, 1], F32, name=nm + "s", tag=nm + "s")
                            nc.scalar.sqrt(rs[:], ri[:])
                            return rs

                        rstd_q = rstd_of(ssq_q, Q_RANK, "rq")
                        rstd_kv = rstd_of(ssq_kv, KV_RANK, "rk")

                        # normalized q latent (bf16) -> 6 transposed chunks
                        qn = dqs.tile([P, Q_RANK], BF16, name="qn", tag="qn")
                        nc.vector.tensor_scalar(qn[:], tmp[:, 0:Q_RANK], rstd_q[:],
                                                None, op0=ALU.mult)
                        for rc in range(6):
                            transpose_to(
                                qnT_all[:, rc * SEQ + sb * P:rc * SEQ + (sb + 1) * P],
                                qn[:, rc * P:(rc + 1) * P], P, P)

                        # normalized kv latent -> kvnT chunks
                        kvn = dqs.tile([P, KV_RANK], BF16, name="kvn", tag="kvn")
                        nc.vector.tensor_scalar(kvn[:],
                                                tmp[:, Q_RANK:Q_RANK + KV_RANK],
                                                rstd_kv[:], None, op0=ALU.mult)
                        for cc in range(4):
                            transpose_to(
                                kvnT[:, cc * SEQ + sb * P:cc * SEQ + (sb + 1) * P],
                                kvn[:, cc * P:(cc + 1) * P], P, P)

                        # k_rot: rope then transpose
                        krp = dqs.tile([P, QK_ROT], BF16, name="krp", tag="krp")
                        rope(krp[:], tmp[:, Q_RANK + KV_RANK:DCAT], sb, 32)
                        transpose_to(krT[:, sb * P:(sb + 1) * P], krp[:], P, QK_ROT)

                # -- phase Q: q_up + rope + transposes per s-block --
                with tc.tile_pool(name="qs2", bufs=2) as qs2, \
                     tc.tile_pool(name="ppqu", bufs=2, space="PSUM") as ppqu:
                    for sb in range(NSB):
                        psq = ppqu.tile([P, NQU], F32, name="psq", tag="psq")
                        for rc in range(6):
                            qk = qnT_all[:, rc * SEQ + sb * P:rc * SEQ + (sb + 1) * P]
                            nc.tensor.matmul(psq[:, 0:512], qk,
                                             wqu_t[:, rc * NQU:rc * NQU + 512],
                                             start=(rc == 0), stop=(rc == 5))
                            nc.tensor.matmul(psq[:, 512:NQU], qk,
                                             wqu_t[:, rc * NQU + 512:(rc + 1) * NQU],
                                             start=(rc == 0), stop=(rc == 5))

                        q_sb = qs2.tile([P, NQU], BF16, name="q_sb", tag="q_sb")
                        nc.vector.tensor_tensor(q_sb[:], psq[:], qub_bc[:],
                                                op=ALU.add)

                        # static parts -> qsT per head
                        for h in range(HPC):
                            transpose_to(
                                qsT[:, h * SEQ + sb * P:h * SEQ + (sb + 1) * P],
                                q_sb[:, h * P:(h + 1) * P], P, P)
                        # rot part: cols [512:640]=lo x4 heads, [640:768]=hi x4
                        qrot = qs2.tile([P, 256], BF16, name="qrot", tag="qrot")
                        rope(qrot[:], q_sb[:, 512:NQU], sb, 128)
                        for h in range(HPC):
                            transpose_to(qrT[h][0:32, sb * P:(sb + 1) * P],
                                         qrot[:, h * 32:(h + 1) * 32], P, 32)
                            transpose_to(qrT[h][32:64, sb * P:(sb + 1) * P],
                                         qrot[:, 128 + h * 32:128 + (h + 1) * 32],
                                         P, 32)

            # ---------- PHASE A0: effective K / transposed V per head ----------
            kvf = ctx.enter_context(tc.tile_pool(name="kvf", bufs=1))
            keff = kvf.tile([P, HPC * SEQ], BF16, tag="keff")
            veffT = kvf.tile([P, HPC * SEQ], BF16, tag="veffT")
            with tc.tile_pool(name="wkv", bufs=1) as wkv, \
                 tc.tile_pool(name="ppa0", bufs=2, space="PSUM") as ppa0, \
                 tc.tile_pool(name="ppav", bufs=4, space="PSUM") as ppav:
                wk_t = wkv.tile([P, HPC * 4 * P], BF16, tag="wk_t")
                nc.sync.dma_start(wk_t[:], wk[:])
                wv_t = wkv.tile([P, HPC * 4 * P], BF16, tag="wv_t")
                nc.sync.dma_start(wv_t[:], wv[:])

                for h in range(HPC):
                    # keff[qk, t] = wk_h^T kvn^T  (accumulate c-chunks)
                    for th in range(2):
                        psk = ppa0.tile([P, 512], F32, name="psk", tag="psk")
                        for cc in range(4):
                            ks = kvnT[:, cc * SEQ + th * 512:cc * SEQ + (th + 1) * 512]
                            nc.tensor.matmul(
                                psk[:],
                                wk_t[:, (h * 4 + cc) * P:(h * 4 + cc + 1) * P],
                                ks, start=(cc == 0), stop=(cc == 3))
                        nc.vector.tensor_copy(
                            keff[:, h * SEQ + th * 512:h * SEQ + (th + 1) * 512],
                            psk[:])
                    # veffT[t, v] = kvn wv_h  (lhsT = kvnT chunk slices)
                    for tb in range(NSB):
                        psv = ppav.tile([P, P], F32, name="psv", tag="psv")
                        for cc in range(4):
                            nc.tensor.matmul(
                                psv[:],
                                kvnT[:, cc * SEQ + tb * P:cc * SEQ + (tb + 1) * P],
                                wv_t[:, (h * 4 + cc) * P:(h * 4 + cc + 1) * P],
                                start=(cc == 0), stop=(cc == 3))
                        nc.vector.tensor_copy(
                            veffT[:, h * SEQ + tb * P:h * SEQ + (tb + 1) * P],
                            psv[:])

            # ---------- PHASE A1: transposed scores + softmax + ctx ----------
            mskp = ctx.enter_context(tc.tile_pool(name="mskp", bufs=1))
            maskT_t = mskp.tile([P, NSB * SEQ], BF16, tag="maskT_t")
            nc.sync.dma_start(maskT_t[:], maskT[:])
            with tc.tile_pool(name="att", bufs=2) as att, \
                 tc.tile_pool(name="ppsc", bufs=2, space="PSUM") as ppsc, \
                 tc.tile_pool(name="pprs", bufs=1, space="PSUM") as pprs, \
                 tc.tile_pool(name="ppcx", bufs=2, space="PSUM") as ppcx:
                for h in range(HPC):
                    probsT = att.tile([P, NSB * SEQ], BF16, name="probsT",
                                      tag="probsT")
                    for tb in range(NSB):
                        ps_sc = ppsc.tile([P, SEQ], F32, name="ps_sc", tag="ps_sc")
                        for sh in range(2):
                            nc.tensor.matmul(
                                ps_sc[:, sh * 512:(sh + 1) * 512],
                                keff[:, h * SEQ + tb * P:h * SEQ + (tb + 1) * P],
                                qsT[:, h * SEQ + sh * 512:h * SEQ + (sh + 1) * 512],
                                start=True, stop=False)
                            nc.tensor.matmul(
                                ps_sc[:, sh * 512:(sh + 1) * 512],
                                krT[:, tb * P:(tb + 1) * P],
                                qrT[h][:, sh * 512:(sh + 1) * 512],
                                start=False, stop=False)
                            nc.tensor.matmul(
                                ps_sc[:, sh * 512:(sh + 1) * 512],
                                ident[:],
                                maskT_t[:, tb * SEQ + sh * 512:tb * SEQ + (sh + 1) * 512],
                                start=False, stop=True)
                            nc.scalar.activation(
                                probsT[:, tb * SEQ + sh * 512:tb * SEQ + (sh + 1) * 512],
                                ps_sc[:, sh * 512:(sh + 1) * 512], AF.Exp)
                    for sh in range(2):
                        # row sums via ones-matmul over t
                        ps_rs = pprs.tile([1, 512], F32, name="ps_rs", tag="ps_rs")
                        for tb in range(NSB):
                            nc.tensor.matmul(
                                ps_rs[:], onesc[:],
                                probsT[:, tb * SEQ + sh * 512:tb * SEQ + (sh + 1) * 512],
                                start=(tb == 0), stop=(tb == NSB - 1))
                        rs_row = scv.tile([1, 512], BF16, name="rs_row", tag="rs_row")
                        nc.vector.tensor_copy(rs_row[:], ps_rs[:])
                        ps_bc2 = pprs.tile([P, 512], F32, name="ps_bc2",
                                           tag="ps_bc2")
                        nc.tensor.matmul(ps_bc2[:], ones1[:], rs_row[:],
                                         start=True, stop=True)
                        rcp_bc = att.tile([P, 512], BF16, name="rcp_bc",
                                          tag="rcp_bc")
                        with nc.allow_low_precision(reason="softmax 1/rowsum bf16"):
                            nc.vector.reciprocal(rcp_bc[:], ps_bc2[:])
                        # ctx (unnormalized) then normalize during evacuation
                        ps_ctx = ppcx.tile([P, 512], F32, name="ps_ctx",
                                           tag="ps_ctx")
                        for tb in range(NSB):
                            nc.tensor.matmul(
                                ps_ctx[:],
                                veffT[:, h * SEQ + tb * P:h * SEQ + (tb + 1) * P],
                                probsT[:, tb * SEQ + sh * 512:tb * SEQ + (sh + 1) * 512],
                                start=(tb == 0), stop=(tb == NSB - 1))
                        nc.vector.tensor_tensor(ctxT[h][:, sh * 512:(sh + 1) * 512],
                                                ps_ctx[:], rcp_bc[:], op=ALU.mult)

            # ---------- PHASE O: output projection (partial) ----------
            with tc.tile_pool(name="wop", bufs=1) as wop, \
                 tc.tile_pool(name="ost", bufs=2) as ost, \
                 tc.tile_pool(name="ppo", bufs=4, space="PSUM") as ppo:
                wo_t = wop.tile([P, HPC * DIM], BF16, tag="wo_t")
                nc.sync.dma_start(wo_t[:], wo[:])

                for sb in range(NSB):
                    ostage = ost.tile([P, DIM], F32, name="ostage", tag="ostage")
                    for nb in range(4):
                        pso = ppo.tile([P, 512], F32, name="pso", tag="pso")
                        for h in range(HPC):
                            nc.tensor.matmul(
                                pso[:], ctxT[h][:, sb * P:(sb + 1) * P],
                                wo_t[:, h * DIM + nb * 512:h * DIM + (nb + 1) * 512],
                                start=(h == 0), stop=(h == HPC - 1))
                        nc.vector.tensor_tensor(ostage[:, nb * 512:(nb + 1) * 512],
                                                pso[:],
                                                ob_bc[:, nb * 512:(nb + 1) * 512],
                                                op=ALU.add)
                    nc.sync.dma_start(out_p[sb * P:(sb + 1) * P, :], ostage[:])

    nc.compile()
    return nc


def prep_core_inputs(x, mask, q_down_w, q_down_b, q_norm_scale, q_up_w, q_up_b,
                     kv_down_w, kv_down_b, kv_norm_scale, kv_up_w, out_w, out_b):
    """Host-side shard/pack prep. Returns list of 8 in_maps (bf16 tiles)."""
    import ml_dtypes
    bf = ml_dtypes.bfloat16
    f = np.float32
    inv = f(1.0 / np.sqrt(QK_TOTAL))

    # fused down-proj: rows q_down (768) + kv_down (576) -> [1344, 2048]
    wcat = np.concatenate([np.asarray(q_down_w, f), np.asarray(kv_down_w, f)], 0)
    wd = np.ascontiguousarray(
        wcat.T.reshape(NKD, P, DCAT).transpose(1, 0, 2).reshape(P, NKD * DCAT)
    ).astype(bf)
    bcat = np.concatenate([np.asarray(q_down_b, f), np.asarray(kv_down_b, f)]
                          ).reshape(1, DCAT).astype(bf)

    # q_up with norm scale and 1/sqrt(qk_total) folded in
    q_up_eff = (np.asarray(q_up_w, f) * np.asarray(q_norm_scale, f)[None, :]) * inv
    q_up_eff = q_up_eff.reshape(H, QK_TOTAL, Q_RANK)
    qub_eff = (np.asarray(q_up_b, f) * inv).reshape(H, QK_TOTAL)

    wk_all = np.asarray(kv_up_w, f)[:H * QK_STATIC].reshape(H, QK_STATIC, KV_RANK)
    wv_all = np.asarray(kv_up_w, f)[-H * V_DIM:].reshape(H, V_DIM, KV_RANK)
    kvs = np.asarray(kv_norm_scale, f)

    # rope tables: cos/sin with the "half-rotate" sign pre-applied
    invf = 1.0 / (10000.0 ** (np.arange(0, QK_ROT, 2, dtype=np.float64) / QK_ROT))
    freqs = np.arange(SEQ, dtype=np.float64)[:, None] * invf[None, :]
    cosf = np.concatenate([np.cos(freqs), np.cos(freqs)], -1).astype(f)  # (S, 64)
    sinf = np.concatenate([np.sin(freqs), np.sin(freqs)], -1).astype(f)
    sinhr = sinf.copy()
    sinhr[:, :QK_ROT // 2] *= -1.0
    # per sb: [cos_lo x4 | cos_hi x4 | sinhr_lo x4 | sinhr_hi x4] (each 128 cols)
    cs4 = np.zeros((P, NSB * 512), f)
    for sb in range(NSB):
        rows = slice(sb * P, (sb + 1) * P)
        cs4[:, sb * 512 + 0:sb * 512 + 128] = np.tile(cosf[rows, 0:32], 4)
        cs4[:, sb * 512 + 128:sb * 512 + 256] = np.tile(cosf[rows, 32:64], 4)
        cs4[:, sb * 512 + 256:sb * 512 + 384] = np.tile(sinhr[rows, 0:32], 4)
        cs4[:, sb * 512 + 384:sb * 512 + 512] = np.tile(sinhr[rows, 32:64], 4)
    cs4 = cs4.astype(bf)

    x = np.asarray(x, f)
    mask = np.asarray(mask, f)
    out_b = np.asarray(out_b, f)
    out_w = np.asarray(out_w, f)

    in_maps = []
    for c in range(N_CORES):
        b, g = c // 4, c % 4
        habs = [g * HPC + h for h in range(HPC)]

        # x^T tiles: [p, (sb, k, j)] = x[b, sb*128+j, k*128+p]
        xtb = np.ascontiguousarray(
            x[b].reshape(NSB, P, NKD, P).transpose(3, 0, 2, 1).reshape(P, NSB * NKD * P)
        ).astype(bf)

        # q_up cols: [h0..h3 static (512) | lo x4 (128) | hi x4 (128)]
        wq_cols = np.zeros((Q_RANK, NQU), f)
        qub_cols = np.zeros((1, NQU), f)
        for h, ha in enumerate(habs):
            wq_cols[:, h * P:(h + 1) * P] = q_up_eff[ha, :QK_STATIC].T
            wq_cols[:, 512 + h * 32:512 + (h + 1) * 32] = q_up_eff[ha, QK_STATIC:QK_STATIC + 32].T
            wq_cols[:, 640 + h * 32:640 + (h + 1) * 32] = q_up_eff[ha, QK_STATIC + 32:].T
            qub_cols[0, h * P:(h + 1) * P] = qub_eff[ha, :QK_STATIC]
            qub_cols[0, 512 + h * 32:512 + (h + 1) * 32] = qub_eff[ha, QK_STATIC:QK_STATIC + 32]
            qub_cols[0, 640 + h * 32:640 + (h + 1) * 32] = qub_eff[ha, QK_STATIC + 32:]
        wqu_p = np.ascontiguousarray(
            wq_cols.reshape(6, P, NQU).transpose(1, 0, 2).reshape(P, 6 * NQU)
        ).astype(bf)

        wk_p = np.zeros((P, HPC * 4 * P), f)
        wv_p = np.zeros((P, HPC * 4 * P), f)
        for h, ha in enumerate(habs):
            wkT = (wk_all[ha] * kvs[None, :]).T          # [c=512, qk=128]
            wvT = (wv_all[ha] * kvs[None, :]).T
            for cc in range(4):
                wk_p[:, (h * 4 + cc) * P:(h * 4 + cc + 1) * P] = wkT[cc * P:(cc + 1) * P]
                wv_p[:, (h * 4 + cc) * P:(h * 4 + cc + 1) * P] = wvT[cc * P:(cc + 1) * P]

        wo_p = np.zeros((P, HPC * DIM), f)
        for h, ha in enumerate(habs):
            wo_p[:, h * DIM:(h + 1) * DIM] = out_w[:, ha * V_DIM:(ha + 1) * V_DIM].T
        ob_p = (out_b if g == 0 else np.zeros_like(out_b)).reshape(1, DIM)

        # mask^T tiles: block tb = mask[b][:, tb*128:(tb+1)*128].T  -> [t, s]
        maskTb = np.ascontiguousarray(
            mask[b].T.reshape(NSB, P, SEQ).transpose(1, 0, 2).reshape(P, NSB * SEQ)
        ).astype(bf)

        in_maps.append({
            "xt": xtb,
            "wd": wd, "bcat": bcat,
            "wqu": wqu_p, "qub": qub_cols.astype(bf),
            "wk": wk_p.astype(bf), "wv": wv_p.astype(bf),
            "wo": wo_p.astype(bf), "ob": ob_p.astype(bf),
            "maskT": maskTb, "cs4": cs4,
        })
    return in_maps


_NC_CACHE = None


def kernel(**inputs):
    global _NC_CACHE
    x = np.asarray(inputs["x"], dtype=np.float32)
    args = {k: np.asarray(v) for k, v in inputs.items()
            if k not in ("x", "start_pos")}
    in_maps = prep_core_inputs(x=x, **{k: args[k] for k in (
        "mask", "q_down_w", "q_down_b", "q_norm_scale", "q_up_w", "q_up_b",
        "kv_down_w", "kv_down_b", "kv_norm_scale", "kv_up_w", "out_w", "out_b")})
    if _NC_CACHE is None:
        _NC_CACHE = build_kernel()
    res = run_bass_kernel_spmd(_NC_CACHE, in_maps, list(range(N_CORES))).results
    out = np.zeros((BS, SEQ, DIM), dtype=np.float32)
    for c in range(N_CORES):
        out[c // 4] += res[c]["out_p"]
    return out


# revision 18
# speedup vs baseline: 1.9475x; 1.0283x over previous
"""MLA (multi-head latent attention) prefill kernel for 8 TRN2 NeuronCores.

Sharding: 4 head-groups x 2 batches. Core c: batch = c // 4, head-group g = c % 4
(4 heads each). Each core computes its batch's down-projections + RMSNorm,
its 4 heads' q_up / attention / ctx, and a partial output projection
(out_w column slice). Host sums the 4 partials per batch (TP unshard).

v3: bf16 operands throughout (FWL, half DMA). Attention computes scores
TRANSPOSED [t, s] directly (both operand orientations already exist), so
the probability matrix never needs transposing: probsT = exp(scoresT +
maskT) raw (scores are O(10), no max subtraction needed), row sums come
from ones-matmuls, and softmax normalization is folded into the ctx PSUM
evacuation via a rank-1 broadcast of 1/rowsum. veffT is computed directly
in transposed form. DVE-based RMSNorm stats; single ACT table swap.
"""

import sys
import os

for _p in ("/opt/trn_rl_repo", "/root/.axon_site/_ro/trn_rl_repo"):
    if os.path.isdir(_p) and _p not in sys.path:
        sys.path.insert(0, _p)

import numpy as np

import concourse.bass as bass
import concourse.bacc as bacc
import concourse.tile as tile
import concourse.mybir as mybir
from concourse.bass_utils import run_bass_kernel_spmd
from concourse.masks import make_identity

F32 = mybir.dt.float32
BF16 = mybir.dt.bfloat16
AF = mybir.ActivationFunctionType
ALU = mybir.AluOpType

DIM, H, Q_RANK, KV_RANK = 2048, 16, 768, 512
QK_STATIC, QK_ROT, V_DIM = 128, 64, 128
QK_TOTAL = QK_STATIC + QK_ROT
BS, SEQ = 2, 1024
HPC = 4          # heads per core
N_CORES = 8
P = 128
NSB = SEQ // P   # 8 s-blocks
NKD = DIM // P   # 16 d-chunks
DCAT = Q_RANK + KV_RANK + QK_ROT   # 1344 fused down-proj output cols
NQU = HPC * QK_TOTAL               # 768 q_up cols for this core


def build_kernel():
    nc = bacc.Bacc("TRN2", target_bir_lowering=False, debug=False)

    def din(name, shape, dt=BF16):
        return nc.dram_tensor(name, list(shape), dt, kind="ExternalInput")

    xt = din("xt", (P, NSB * NKD * P))           # x^T tiles per (sb, k)
    wd = din("wd", (P, NKD * DCAT))              # fused down-proj weights
    bcat = din("bcat", (1, DCAT))                # fused down-proj bias row
    wqu = din("wqu", (P, 6 * NQU))               # q_up weights (6 r-chunks)
    qub = din("qub", (1, NQU))                   # q_up bias row
    wk = din("wk", (P, HPC * 4 * P))             # absorbed key weights
    wv = din("wv", (P, HPC * 4 * P))             # absorbed value weights
    wo = din("wo", (P, HPC * DIM))               # out-proj slice
    ob = din("ob", (1, DIM))                     # out bias row (core g==0)
    maskT = din("maskT", (P, NSB * SEQ))         # mask^T tiles per t-block
    cs4 = din("cs4", (P, NSB * 512))             # rope tables per sb (x4 heads)

    out_p = nc.dram_tensor("out_p", [SEQ, DIM], F32, kind="ExternalOutput")

    with tile.TileContext(nc) as tc:
        import contextlib
        ctx = contextlib.ExitStack()
        with ctx:
            const = ctx.enter_context(tc.tile_pool(name="const", bufs=1))
            pers = ctx.enter_context(tc.tile_pool(name="pers", bufs=1))
            scv = ctx.enter_context(tc.tile_pool(name="scv", bufs=4))

            ident = const.tile([P, P], BF16, tag="ident")
            make_identity(nc, ident[:])
            ones1 = const.tile([1, P], BF16, tag="ones1")
            nc.gpsimd.memset(ones1[:], 1.0)
            onesc = const.tile([P, 1], BF16, tag="onesc")
            nc.gpsimd.memset(onesc[:], 1.0)

            t_cs4 = const.tile([P, NSB * 512], BF16, tag="cs4")
            nc.sync.dma_start(t_cs4[:], cs4[:])
            t_bcat = const.tile([1, DCAT], BF16, tag="bcat")
            nc.sync.dma_start(t_bcat[:], bcat[:])
            t_qub = const.tile([1, NQU], BF16, tag="qub")
            nc.sync.dma_start(t_qub[:], qub[:])
            t_ob = const.tile([1, DIM], BF16, tag="ob")
            nc.sync.dma_start(t_ob[:], ob[:])

            # persistent activations
            kvnT = pers.tile([P, 4 * SEQ], BF16, tag="kvnT")        # 4 c-chunks
            krT = pers.tile([64, SEQ], BF16, tag="krT")
            qnT_all = pers.tile([P, 6 * SEQ], BF16, tag="qnT_all")  # 6 r-chunks
            qsT = pers.tile([P, HPC * SEQ], BF16, tag="qsT")        # per head
            qrT = [pers.tile([64, SEQ], BF16, name=f"qrT{h}", tag=f"qrT{h}")
                   for h in range(HPC)]
            ctxT = [pers.tile([P, SEQ], BF16, name=f"ctxT{h}", tag=f"ctxT{h}")
                    for h in range(HPC)]

            # broadcast bias tiles (bias value replicated down partitions)
            bias_bc = pers.tile([P, DCAT], F32, tag="bias_bc")
            qub_bc = pers.tile([P, NQU], F32, tag="qub_bc")
            ob_bc = pers.tile([P, DIM], F32, tag="ob_bc")
            with tc.tile_pool(name="ppbc", bufs=2, space="PSUM") as ppbc:
                for dst, src, w in ((bias_bc, t_bcat, DCAT), (qub_bc, t_qub, NQU),
                                    (ob_bc, t_ob, DIM)):
                    for n0 in range(0, w, 512):
                        n1 = min(n0 + 512, w)
                        psb = ppbc.tile([P, 512], F32, name="ps_bc", tag="ps_bc")
                        nc.tensor.matmul(psb[:, 0:n1 - n0], ones1[:], src[:, n0:n1],
                                         start=True, stop=True)
                        nc.vector.tensor_copy(dst[:, n0:n1], psb[:, 0:n1 - n0])

            # prefetch attention/out-proj weights + mask early (overlap phase D)
            wkv = ctx.enter_context(tc.tile_pool(name="wkv", bufs=1))
            wk_t = wkv.tile([P, HPC * 4 * P], BF16, tag="wk_t")
            nc.sync.dma_start(wk_t[:], wk[:])
            wv_t = wkv.tile([P, HPC * 4 * P], BF16, tag="wv_t")
            nc.sync.dma_start(wv_t[:], wv[:])
            mskp = ctx.enter_context(tc.tile_pool(name="mskp", bufs=1))
            maskT_t = mskp.tile([P, NSB * SEQ], BF16, tag="maskT_t")
            nc.sync.dma_start(maskT_t[:], maskT[:])
            wop = ctx.enter_context(tc.tile_pool(name="wop", bufs=1))
            wo_t = wop.tile([P, HPC * DIM], BF16, tag="wo_t")
            nc.sync.dma_start(wo_t[:], wo[:])

            def rope(dst_ap, src_ap, sb, width):
                # dst/src: [P, 2*width]; tables: cos_lo|cos_hi|sinhr_lo|sinhr_hi
                cb = sb * 512
                c_lo = t_cs4[:, cb:cb + width]
                c_hi = t_cs4[:, cb + 128:cb + 128 + width]
                s_lo = t_cs4[:, cb + 256:cb + 256 + width]
                s_hi = t_cs4[:, cb + 384:cb + 384 + width]
                m1 = scv.tile([P, 128], BF16, name="rp1", tag="rp1")
                m2 = scv.tile([P, 128], BF16, name="rp2", tag="rp2")
                lo, hi = src_ap[:, 0:width], src_ap[:, width:2 * width]
                nc.gpsimd.tensor_tensor(m1[:, 0:width], lo, c_lo, op=ALU.mult)
                nc.gpsimd.tensor_tensor(m2[:, 0:width], hi, s_lo, op=ALU.mult)
                nc.gpsimd.tensor_tensor(dst_ap[:, 0:width], m1[:, 0:width],
                                        m2[:, 0:width], op=ALU.add)
                nc.gpsimd.tensor_tensor(m1[:, 0:width], hi, c_hi, op=ALU.mult)
                nc.gpsimd.tensor_tensor(m2[:, 0:width], lo, s_hi, op=ALU.mult)
                nc.gpsimd.tensor_tensor(dst_ap[:, width:2 * width], m1[:, 0:width],
                                        m2[:, 0:width], op=ALU.add)

            # ---------- PHASES D + Q: down-proj, norm, q_up ----------
            with tc.tile_pool(name="wdq", bufs=1) as wdq, \
                 tc.tile_pool(name="ppt", bufs=2, space="PSUM") as ppt:

                def transpose_to(dst_ap, src_ap, rows, cols):
                    pst = ppt.tile([P, P], BF16, name="tr", tag="tr")
                    nc.tensor.transpose(pst[0:cols, 0:rows], src_ap,
                                        ident[:rows, :rows])
                    nc.vector.tensor_copy(dst_ap, pst[0:cols, 0:rows])

                wd_t = wdq.tile([P, NKD * DCAT], BF16, tag="wd_t")
                for q in range(8):
                    nc.sync.dma_start(wd_t[:, q * 2 * DCAT:(q + 1) * 2 * DCAT],
                                      wd[:, q * 2 * DCAT:(q + 1) * 2 * DCAT])
                wqu_t = wdq.tile([P, 6 * NQU], BF16, tag="wqu_t")
                nc.sync.dma_start(wqu_t[:], wqu[:])

                # -- phase D: fused down-proj + RMSNorm per s-block --
                with tc.tile_pool(name="xs", bufs=2) as xs_pool, \
                     tc.tile_pool(name="dqs", bufs=2) as dqs, \
                     tc.tile_pool(name="ppdq", bufs=2, space="PSUM") as ppdq:
                    for sb in range(NSB):
                        x_sb = xs_pool.tile([P, NKD * P], BF16, name="x_sb",
                                            tag="x_sb")
                        nc.sync.dma_start(x_sb[:],
                                          xt[:, sb * NKD * P:(sb + 1) * NKD * P])

                        ps = ppdq.tile([P, DCAT], F32, name="psd", tag="psd")
                        for k in range(NKD):
                            xk = x_sb[:, k * P:(k + 1) * P]
                            wb = k * DCAT
                            nc.tensor.matmul(ps[:, 0:512], xk, wd_t[:, wb:wb + 512],
                                             start=(k == 0), stop=(k == NKD - 1))
                            nc.tensor.matmul(ps[:, 512:1024], xk,
                                             wd_t[:, wb + 512:wb + 1024],
                                             start=(k == 0), stop=(k == NKD - 1))
                            nc.tensor.matmul(ps[:, 1024:1344], xk,
                                             wd_t[:, wb + 1024:wb + 1344],
                                             start=(k == 0), stop=(k == NKD - 1))

                        # bias add into fp32 scratch (also the norm input)
                        tmp = dqs.tile([P, DCAT], F32, name="tmp", tag="tmp")
                        nc.vector.tensor_tensor(tmp[:], ps[:], bias_bc[:],
                                                op=ALU.add)

                        # RMSNorm stats (DVE square+reduce, ACT sqrt)
                        sqf = dqs.tile([P, Q_RANK + KV_RANK], BF16,
                                       name="sqf", tag="sqf")
                        ssq_q = scv.tile([P, 1], F32, name="ssq_q", tag="ssq_q")
                        ssq_kv = scv.tile([P, 1], F32, name="ssq_kv", tag="ssq_kv")
                        nc.gpsimd.tensor_tensor(sqf[:], tmp[:, 0:Q_RANK + KV_RANK],
                                                tmp[:, 0:Q_RANK + KV_RANK],
                                                op=ALU.mult)
                        nc.vector.tensor_reduce(ssq_q[:], sqf[:, 0:Q_RANK],
                                                axis=mybir.AxisListType.X,
                                                op=ALU.add)
                        nc.vector.tensor_reduce(ssq_kv[:], sqf[:, Q_RANK:],
                                                axis=mybir.AxisListType.X,
                                                op=ALU.add)

                        def rstd_of(ssq, n, nm):
                            ms = scv.tile([P, 1], F32, name=nm + "m", tag=nm + "m")
                            nc.vector.tensor_scalar(ms[:], ssq[:], 1.0 / n, 1e-6,
                                                    op0=ALU.mult, op1=ALU.add)
                            ri = scv.tile([P, 1], F32, name=nm + "i", tag=nm + "i")
                            nc.vector.reciprocal(ri[:], ms[:])
                            rs = scv.tile([P, 1], F32, name=nm + "s", tag=nm + "s")
                            nc.scalar.sqrt(rs[:], ri[:])
                            return rs

                        rstd_q = rstd_of(ssq_q, Q_RANK, "rq")
                        rstd_kv = rstd_of(ssq_kv, KV_RANK, "rk")

                        # normalized q latent (bf16) -> 6 transposed chunks
                        qn = dqs.tile([P, Q_RANK], BF16, name="qn", tag="qn")
                        nc.vector.tensor_scalar(qn[:], tmp[:, 0:Q_RANK], rstd_q[:],
                                                None, op0=ALU.mult)
                        for rc in range(6):
                            transpose_to(
                                qnT_all[:, rc * SEQ + sb * P:rc * SEQ + (sb + 1) * P],
                                qn[:, rc * P:(rc + 1) * P], P, P)

                        # normalized kv latent -> kvnT chunks
                        kvn = dqs.tile([P, KV_RANK], BF16, name="kvn", tag="kvn")
                        nc.vector.tensor_scalar(kvn[:],
                                                tmp[:, Q_RANK:Q_RANK + KV_RANK],
                                                rstd_kv[:], None, op0=ALU.mult)
                        for cc in range(4):
                            transpose_to(
                                kvnT[:, cc * SEQ + sb * P:cc * SEQ + (sb + 1) * P],
                                kvn[:, cc * P:(cc + 1) * P], P, P)

                        # k_rot: rope then transpose
                        krp = dqs.tile([P, QK_ROT], BF16, name="krp", tag="krp")
                        rope(krp[:], tmp[:, Q_RANK + KV_RANK:DCAT], sb, 32)
                        transpose_to(krT[:, sb * P:(sb + 1) * P], krp[:], P, QK_ROT)

                # -- phase Q: q_up + rope + transposes per s-block --
                with tc.tile_pool(name="qs2", bufs=2) as qs2, \
                     tc.tile_pool(name="ppqu", bufs=2, space="PSUM") as ppqu:
                    for sb in range(NSB):
                        psq = ppqu.tile([P, NQU], F32, name="psq", tag="psq")
                        for rc in range(6):
                            qk = qnT_all[:, rc * SEQ + sb * P:rc * SEQ + (sb + 1) * P]
                            nc.tensor.matmul(psq[:, 0:512], qk,
                                             wqu_t[:, rc * NQU:rc * NQU + 512],
                                             start=(rc == 0), stop=(rc == 5))
                            nc.tensor.matmul(psq[:, 512:NQU], qk,
                                             wqu_t[:, rc * NQU + 512:(rc + 1) * NQU],
                                             start=(rc == 0), stop=(rc == 5))

                        q_sb = qs2.tile([P, NQU], BF16, name="q_sb", tag="q_sb")
                        nc.vector.tensor_tensor(q_sb[:], psq[:], qub_bc[:],
                                                op=ALU.add)

                        # static parts -> qsT per head
                        for h in range(HPC):
                            transpose_to(
                                qsT[:, h * SEQ + sb * P:h * SEQ + (sb + 1) * P],
                                q_sb[:, h * P:(h + 1) * P], P, P)
                        # rot part: cols [512:640]=lo x4 heads, [640:768]=hi x4
                        qrot = qs2.tile([P, 256], BF16, name="qrot", tag="qrot")
                        rope(qrot[:], q_sb[:, 512:NQU], sb, 128)
                        for h in range(HPC):
                            transpose_to(qrT[h][0:32, sb * P:(sb + 1) * P],
                                         qrot[:, h * 32:(h + 1) * 32], P, 32)
                            transpose_to(qrT[h][32:64, sb * P:(sb + 1) * P],
                                         qrot[:, 128 + h * 32:128 + (h + 1) * 32],
                                         P, 32)

            # ---------- PHASE A0: effective K / transposed V per head ----------
            kvf = ctx.enter_context(tc.tile_pool(name="kvf", bufs=1))
            keff = kvf.tile([P, HPC * SEQ], BF16, tag="keff")
            veffT = kvf.tile([P, HPC * SEQ], BF16, tag="veffT")
            with tc.tile_pool(name="ppa0", bufs=2, space="PSUM") as ppa0, \
                 tc.tile_pool(name="ppav", bufs=4, space="PSUM") as ppav:
                for h in range(HPC):
                    # keff[qk, t] = wk_h^T kvn^T  (accumulate c-chunks)
                    for th in range(2):
                        psk = ppa0.tile([P, 512], F32, name="psk", tag="psk")
                        for cc in range(4):
                            ks = kvnT[:, cc * SEQ + th * 512:cc * SEQ + (th + 1) * 512]
                            nc.tensor.matmul(
                                psk[:],
                                wk_t[:, (h * 4 + cc) * P:(h * 4 + cc + 1) * P],
                                ks, start=(cc == 0), stop=(cc == 3))
                        nc.vector.tensor_copy(
                            keff[:, h * SEQ + th * 512:h * SEQ + (th + 1) * 512],
                            psk[:])
                    # veffT[t, v] = kvn wv_h  (lhsT = kvnT chunk slices)
                    for tb in range(NSB):
                        psv = ppav.tile([P, P], F32, name="psv", tag="psv")
                        for cc in range(4):
                            nc.tensor.matmul(
                                psv[:],
                                kvnT[:, cc * SEQ + tb * P:cc * SEQ + (tb + 1) * P],
                                wv_t[:, (h * 4 + cc) * P:(h * 4 + cc + 1) * P],
                                start=(cc == 0), stop=(cc == 3))
                        nc.vector.tensor_copy(
                            veffT[:, h * SEQ + tb * P:h * SEQ + (tb + 1) * P],
                            psv[:])

            # ---------- PHASE A1: transposed scores + softmax + ctx ----------
            with tc.tile_pool(name="att", bufs=2) as att, \
                 tc.tile_pool(name="ppsc", bufs=2, space="PSUM") as ppsc, \
                 tc.tile_pool(name="pprs", bufs=1, space="PSUM") as pprs, \
                 tc.tile_pool(name="ppcx", bufs=2, space="PSUM") as ppcx:
                for h in range(HPC):
                    probsT = att.tile([P, NSB * SEQ], BF16, name="probsT",
                                      tag="probsT")
                    for tb in range(NSB):
                        ps_sc = ppsc.tile([P, SEQ], F32, name="ps_sc", tag="ps_sc")
                        for sh in range(2):
                            nc.tensor.matmul(
                                ps_sc[:, sh * 512:(sh + 1) * 512],
                                keff[:, h * SEQ + tb * P:h * SEQ + (tb + 1) * P],
                                qsT[:, h * SEQ + sh * 512:h * SEQ + (sh + 1) * 512],
                                start=True, stop=False)
                            nc.tensor.matmul(
                                ps_sc[:, sh * 512:(sh + 1) * 512],
                                krT[:, tb * P:(tb + 1) * P],
                                qrT[h][:, sh * 512:(sh + 1) * 512],
                                start=False, stop=False)
                            nc.tensor.matmul(
                                ps_sc[:, sh * 512:(sh + 1) * 512],
                                ident[:],
                                maskT_t[:, tb * SEQ + sh * 512:tb * SEQ + (sh + 1) * 512],
                                start=False, stop=True)
                            nc.scalar.activation(
                                probsT[:, tb * SEQ + sh * 512:tb * SEQ + (sh + 1) * 512],
                                ps_sc[:, sh * 512:(sh + 1) * 512], AF.Exp)
                    for sh in range(2):
                        # row sums via ones-matmul over t
                        ps_rs = pprs.tile([1, 512], F32, name="ps_rs", tag="ps_rs")
                        for tb in range(NSB):
                            nc.tensor.matmul(
                                ps_rs[:], onesc[:],
                                probsT[:, tb * SEQ + sh * 512:tb * SEQ + (sh + 1) * 512],
                                start=(tb == 0), stop=(tb == NSB - 1))
                        rs_row = scv.tile([1, 512], BF16, name="rs_row", tag="rs_row")
                        nc.vector.tensor_copy(rs_row[:], ps_rs[:])
                        ps_bc2 = pprs.tile([P, 512], F32, name="ps_bc2",
                                           tag="ps_bc2")
                        nc.tensor.matmul(ps_bc2[:], ones1[:], rs_row[:],
                                         start=True, stop=True)
                        rcp_bc = att.tile([P, 512], BF16, name="rcp_bc",
                                          tag="rcp_bc")
                        with nc.allow_low_precision(reason="softmax 1/rowsum bf16"):
                            nc.vector.reciprocal(rcp_bc[:], ps_bc2[:])
                        # ctx (unnormalized) then normalize during evacuation
                        ps_ctx = ppcx.tile([P, 512], F32, name="ps_ctx",
                                           tag="ps_ctx")
                        for tb in range(NSB):
                            nc.tensor.matmul(
                                ps_ctx[:],
                                veffT[:, h * SEQ + tb * P:h * SEQ + (tb + 1) * P],
                                probsT[:, tb * SEQ + sh * 512:tb * SEQ + (sh + 1) * 512],
                                start=(tb == 0), stop=(tb == NSB - 1))
                        nc.vector.tensor_tensor(ctxT[h][:, sh * 512:(sh + 1) * 512],
                                                ps_ctx[:], rcp_bc[:], op=ALU.mult)

            # ---------- PHASE O: output projection (partial) ----------
            with tc.tile_pool(name="ost", bufs=2) as ost, \
                 tc.tile_pool(name="ppo", bufs=4, space="PSUM") as ppo:
                for sb in range(NSB):
                    ostage = ost.tile([P, DIM], F32, name="ostage", tag="ostage")
                    for nb in range(4):
                        pso = ppo.tile([P, 512], F32, name="pso", tag="pso")
                        for h in range(HPC):
                            nc.tensor.matmul(
                                pso[:], ctxT[h][:, sb * P:(sb + 1) * P],
                                wo_t[:, h * DIM + nb * 512:h * DIM + (nb + 1) * 512],
                                start=(h == 0), stop=(h == HPC - 1))
                        nc.vector.tensor_tensor(ostage[:, nb * 512:(nb + 1) * 512],
                                                pso[:],
                                                ob_bc[:, nb * 512:(nb + 1) * 512],
                                                op=ALU.add)
                    nc.sync.dma_start(out_p[sb * P:(sb + 1) * P, :], ostage[:])

    nc.compile()
    return nc


def prep_core_inputs(x, mask, q_down_w, q_down_b, q_norm_scale, q_up_w, q_up_b,
                     kv_down_w, kv_down_b, kv_norm_scale, kv_up_w, out_w, out_b):
    """Host-side shard/pack prep. Returns list of 8 in_maps (bf16 tiles)."""
    import ml_dtypes
    bf = ml_dtypes.bfloat16
    f = np.float32
    inv = f(1.0 / np.sqrt(QK_TOTAL))

    # fused down-proj: rows q_down (768) + kv_down (576) -> [1344, 2048]
    wcat = np.concatenate([np.asarray(q_down_w, f), np.asarray(kv_down_w, f)], 0)
    wd = np.ascontiguousarray(
        wcat.T.reshape(NKD, P, DCAT).transpose(1, 0, 2).reshape(P, NKD * DCAT)
    ).astype(bf)
    bcat = np.concatenate([np.asarray(q_down_b, f), np.asarray(kv_down_b, f)]
                          ).reshape(1, DCAT).astype(bf)

    # q_up with norm scale and 1/sqrt(qk_total) folded in
    q_up_eff = (np.asarray(q_up_w, f) * np.asarray(q_norm_scale, f)[None, :]) * inv
    q_up_eff = q_up_eff.reshape(H, QK_TOTAL, Q_RANK)
    qub_eff = (np.asarray(q_up_b, f) * inv).reshape(H, QK_TOTAL)

    wk_all = np.asarray(kv_up_w, f)[:H * QK_STATIC].reshape(H, QK_STATIC, KV_RANK)
    wv_all = np.asarray(kv_up_w, f)[-H * V_DIM:].reshape(H, V_DIM, KV_RANK)
    kvs = np.asarray(kv_norm_scale, f)

    # rope tables: cos/sin with the "half-rotate" sign pre-applied
    invf = 1.0 / (10000.0 ** (np.arange(0, QK_ROT, 2, dtype=np.float64) / QK_ROT))
    freqs = np.arange(SEQ, dtype=np.float64)[:, None] * invf[None, :]
    cosf = np.concatenate([np.cos(freqs), np.cos(freqs)], -1).astype(f)  # (S, 64)
    sinf = np.concatenate([np.sin(freqs), np.sin(freqs)], -1).astype(f)
    sinhr = sinf.copy()
    sinhr[:, :QK_ROT // 2] *= -1.0
    # per sb: [cos_lo x4 | cos_hi x4 | sinhr_lo x4 | sinhr_hi x4] (each 128 cols)
    cs4 = np.zeros((P, NSB * 512), f)
    for sb in range(NSB):
        rows = slice(sb * P, (sb + 1) * P)
        cs4[:, sb * 512 + 0:sb * 512 + 128] = np.tile(cosf[rows, 0:32], 4)
        cs4[:, sb * 512 + 128:sb * 512 + 256] = np.tile(cosf[rows, 32:64], 4)
        cs4[:, sb * 512 + 256:sb * 512 + 384] = np.tile(sinhr[rows, 0:32], 4)
        cs4[:, sb * 512 + 384:sb * 512 + 512] = np.tile(sinhr[rows, 32:64], 4)
    cs4 = cs4.astype(bf)

    x = np.asarray(x, f)
    mask = np.asarray(mask, f)
    out_b = np.asarray(out_b, f)
    out_w = np.asarray(out_w, f)

    in_maps = []
    for c in range(N_CORES):
        b, g = c // 4, c % 4
        habs = [g * HPC + h for h in range(HPC)]

        # x^T tiles: [p, (sb, k, j)] = x[b, sb*128+j, k*128+p]
        xtb = np.ascontiguousarray(
            x[b].reshape(NSB, P, NKD, P).transpose(3, 0, 2, 1).reshape(P, NSB * NKD * P)
        ).astype(bf)

        # q_up cols: [h0..h3 static (512) | lo x4 (128) | hi x4 (128)]
        wq_cols = np.zeros((Q_RANK, NQU), f)
        qub_cols = np.zeros((1, NQU), f)
        for h, ha in enumerate(habs):
            wq_cols[:, h * P:(h + 1) * P] = q_up_eff[ha, :QK_STATIC].T
            wq_cols[:, 512 + h * 32:512 + (h + 1) * 32] = q_up_eff[ha, QK_STATIC:QK_STATIC + 32].T
            wq_cols[:, 640 + h * 32:640 + (h + 1) * 32] = q_up_eff[ha, QK_STATIC + 32:].T
            qub_cols[0, h * P:(h + 1) * P] = qub_eff[ha, :QK_STATIC]
            qub_cols[0, 512 + h * 32:512 + (h + 1) * 32] = qub_eff[ha, QK_STATIC:QK_STATIC + 32]
            qub_cols[0, 640 + h * 32:640 + (h + 1) * 32] = qub_eff[ha, QK_STATIC + 32:]
        wqu_p = np.ascontiguousarray(
            wq_cols.reshape(6, P, NQU).transpose(1, 0, 2).reshape(P, 6 * NQU)
        ).astype(bf)

        wk_p = np.zeros((P, HPC * 4 * P), f)
        wv_p = np.zeros((P, HPC * 4 * P), f)
        for h, ha in enumerate(habs):
            wkT = (wk_all[ha] * kvs[None, :]).T          # [c=512, qk=128]
            wvT = (wv_all[ha] * kvs[None, :]).T
            for cc in range(4):
                wk_p[:, (h * 4 + cc) * P:(h * 4 + cc + 1) * P] = wkT[cc * P:(cc + 1) * P]
                wv_p[:, (h * 4 + cc) * P:(h * 4 + cc + 1) * P] = wvT[cc * P:(cc + 1) * P]

        wo_p = np.zeros((P, HPC * DIM), f)
        for h, ha in enumerate(habs):
            wo_p[:, h * DIM:(h + 1) * DIM] = out_w[:, ha * V_DIM:(ha + 1) * V_DIM].T
        ob_p = (out_b if g == 0 else np.zeros_like(out_b)).reshape(1, DIM)

        # mask^T tiles: block tb = mask[b][:, tb*128:(tb+1)*128].T  -> [t, s]
        maskTb = np.ascontiguousarray(
            mask[b].T.reshape(NSB, P, SEQ).transpose(1, 0, 2).reshape(P, NSB * SEQ)
        ).astype(bf)

        in_maps.append({
            "xt": xtb,
            "wd": wd, "bcat": bcat,
            "wqu": wqu_p, "qub": qub_cols.astype(bf),
            "wk": wk_p.astype(bf), "wv": wv_p.astype(bf),
            "wo": wo_p.astype(bf), "ob": ob_p.astype(bf),
            "maskT": maskTb, "cs4": cs4,
        })
    return in_maps


_NC_CACHE = None


def kernel(**inputs):
    global _NC_CACHE
    x = np.asarray(inputs["x"], dtype=np.float32)
    args = {k: np.asarray(v) for k, v in inputs.items()
            if k not in ("x", "start_pos")}
    in_maps = prep_core_inputs(x=x, **{k: args[k] for k in (
        "mask", "q_down_w", "q_down_b", "q_norm_scale", "q_up_w", "q_up_b",
        "kv_down_w", "kv_down_b", "kv_norm_scale", "kv_up_w", "out_w", "out_b")})
    if _NC_CACHE is None:
        _NC_CACHE = build_kernel()
    res = run_bass_kernel_spmd(_NC_CACHE, in_maps, list(range(N_CORES))).results
    out = np.zeros((BS, SEQ, DIM), dtype=np.float32)
    for c in range(N_CORES):
        out[c // 4] += res[c]["out_p"]
    return out
